# revision 16
# baseline (speedup 1.0000x reference)
"""GQA kernel for Trainium2, 8 NeuronCores, tensor-parallel over heads.

Problem: B=1, T=2048, C=4096, 32 q-heads, 16 kv-heads, head_dim=128,
scale = 1/sqrt(32), causal. q head H uses kv head H%16.

Sharding (no collectives needed): core c owns q-heads
{2c, 2c+1, 2c+16, 2c+17} and kv-heads {2c, 2c+1}. Each output column
block depends only on its own head, so the full output is a host-side
concat of per-core column slices.

Per-core kernel (all matmuls bf16, fp32 PSUM accumulation):
  xT resident in SBUF as [C=4096 (32 chunks of 128 part), T=2048].
  qT[h] = Wq_h @ xT    -> [128 (D), 2048 (T)]   (4 heads)
  kT[kv] = Wk_kv @ xT  -> [128 (D), 2048 (T)]   (2 kv heads)
  v[kv]  = x @ Wv_kv^T -> [2048 (T part), 128+1] (ones col for row sums)
  S^T tile (per 128-wide Tk tile j) = kT_j^T @ qT_block -> [128,512] PSUM
  P^T = exp(SCALE * S^T) (ACT), causal mask via {0,1} multiply on the
        true-diagonal 128-wide tile only; diagonal tiles narrowed to
        the live Tq columns.
  out[Tq,128] (+ row-sum col) = sum_j P^T_j.T @ v_j  (PSUM accum,
        4 Tq-chunks of 128 packed 2-per-bank, zeroed by a PE matmul)
  out normalized by reciprocal(row sum) (DVE), DMA'd out fp32.

Schedule (the big idea vs the 335us version): attention is paced by
the Scalar/ACT engine's exp (~21.6us per head vs ~19us of PE work),
and projections are paced by the PE with ACT idle. Interleaving them
hides the exp entirely: all post-phase-A projection strips are chopped
into 4-kc (0.85us) filler units and pumped into the PE queue between
each attention j-tile's S and PV, via one global filler queue with
forced drains at dependency points (block (h,b) forces its qt/kt/vt
producers). The 150-matmul HAM warmup is gone: phase A's real matmuls
start at first-DMA-arrival and absorb the clock ramp.
"""

import numpy as np
import ml_dtypes

BF16 = ml_dtypes.bfloat16
T = 2048
C = 4096
D = 128
N_HEADS = 32
N_KV = 16
SCALE = float(1.0 / np.sqrt(np.float32(N_HEADS)))
KC = C // 128          # 32 contraction chunks
NQH = 4                # local q heads per core
NKV = 2                # local kv heads per core
NT = T // 128          # 16 token tiles
VROW = D + 1           # 129: v with ones column
N_CORES = 8

_prog_cache = {}


def _build_program():
    if "nc" in _prog_cache:
        return _prog_cache["nc"]
    import concourse.bass as bass
    import concourse.tile as tile
    from concourse import bacc, mybir

    dt = mybir.dt
    f32 = dt.float32
    bf16 = dt.bfloat16
    EXP = mybir.ActivationFunctionType.Exp
    COPY = mybir.ActivationFunctionType.Copy

    nc = bacc.Bacc("TRN2", target_bir_lowering=False, debug=False,
                   num_devices=N_CORES)

    xT_d = nc.dram_tensor("xT", [128, KC * T], bf16, kind="ExternalInput").ap()
    wq_d = nc.dram_tensor("wq", [NQH, 128, C], bf16, kind="ExternalInput").ap()
    wk_d = nc.dram_tensor("wk", [NKV, 128, C], bf16, kind="ExternalInput").ap()
    wv_d = nc.dram_tensor("wv", [NKV, 128, C], bf16, kind="ExternalInput").ap()
    # masks: [128,128] causal diag tile + [128,128] identity for PE transpose
    mask_d = nc.dram_tensor("masks", [128, 256], bf16,
                            kind="ExternalInput").ap()
    out_d = nc.dram_tensor("out", [T, NQH * D], f32, kind="ExternalOutput").ap()

    with tile.TileContext(nc) as tc:
        with (
            tc.tile_pool(name="persist", bufs=1) as persist,
            tc.tile_pool(name="xpool", bufs=1) as xpool,
            tc.tile_pool(name="wpool", bufs=3) as wpool,
            tc.tile_pool(name="ptpool", bufs=3) as ptpool,
            tc.tile_pool(name="opool", bufs=4) as opool,
            tc.tile_pool(name="recpool", bufs=4) as recpool,
            # PSUM: 8 banks = pv 4x[128,512] + spp 2x[128,512] + proj 2x[128,512]
            tc.tile_pool(name="psum", bufs=1, space=bass.MemorySpace.PSUM) as psum,
        ):
            mask_sb = persist.tile([128, 256], bf16, name="mask_sb",
                                   tag="mask_sb")
            tri = mask_sb[:, 0:128]
            ident = mask_sb[:, 128:256]

            qt = persist.tile([128, NQH * T], bf16, name="qt", tag="qt")
            kt = persist.tile([128, NKV * T], bf16, name="kt", tag="kt")
            vt = persist.tile([128, NKV * NT * VROW], bf16, name="vt", tag="vt")
            zer = persist.tile([128, 128], bf16, name="zer", tag="zer")
            nc.vector.memset(zer[:], 0.0)

            # ones columns of v (row-sum trick)
            for i in range(NKV * NT):
                nc.vector.memset(vt[:, i * VROW + D: (i + 1) * VROW], 1.0)

            # xT is DMA'd as 32 per-kc chunks [128, T] so the startup
            # projections can consume chunks at DMA arrival pace.
            xts = [None] * KC
            wts = {}

            def dma_w(src, idx, key):
                w = wpool.tile([128, C], bf16, name=f"w_{key}", tag="w")
                nc.sync.dma_start(out=w[:], in_=src[idx])
                wts[key] = w

            def dma_x(kc):
                xt = xpool.tile([128, T], bf16, name=f"xt{kc}", tag=f"xt{kc}")
                nc.sync.dma_start(out=xt[:], in_=xT_d[:, kc * T:(kc + 1) * T])
                xts[kc] = xt

            def xs(kc, lo, size):
                return xts[kc][:, lo: lo + size]

            def ps_tile(tag, name, shape=(128, 512), dtyp=f32, bufs=None):
                return psum.tile(list(shape), dtyp, name=name, tag=tag,
                                 bufs=bufs)

            def proj_phase_a():
                """q0+k0 strips, kc-outer so program order matches x-chunk
                DMA arrival: PE does 8 matmuls (4.1K cycles) per 0.5MB chunk
                (~1.2us DMA), staying busy through the whole x load. Runs
                straight out of reset: the first couple kc's absorb the HAM
                clock ramp while the DMA stream is still the pacer."""
                wq = wts.pop("q0")
                wk = wts.pop("k0")
                psq = [ps_tile("pv", f"psA_q{t}", bufs=4)[:] for t in range(4)]
                psk = ([ps_tile("proj", f"psA_k{t}", bufs=2)[:]
                        for t in range(2)]
                       + [ps_tile("spp", f"psA_k{t + 2}", bufs=2)[:]
                          for t in range(2)])
                with nc.named_scope("phaseA"):
                    for kc in range(KC):
                        for t4 in range(4):
                            nc.tensor.matmul(
                                psq[t4], lhsT=wq[:, kc * 128:(kc + 1) * 128],
                                rhs=xs(kc, t4 * 512, 512),
                                start=(kc == 0), stop=(kc == KC - 1))
                        for t4 in range(4):
                            nc.tensor.matmul(
                                psk[t4], lhsT=wk[:, kc * 128:(kc + 1) * 128],
                                rhs=xs(kc, t4 * 512, 512),
                                start=(kc == 0), stop=(kc == KC - 1))
                    # cast order: k t4=0,1 first (frees the proj bufs for the
                    # v0 strip), then q (unblocks attn0's S), then k t4=2,3
                    # (frees the spp bufs). Alternate DVE/ACT to halve the
                    # chain.
                    def cast(dst, src, on_act):
                        if on_act:
                            nc.scalar.copy(out=dst, in_=src)
                        else:
                            nc.vector.tensor_copy(out=dst, in_=src)

                    cast(kt[:, 0:512], psk[0], False)
                    cast(kt[:, 512:1024], psk[1], True)
                    for t4 in range(4):
                        cast(qt[:, t4 * 512:(t4 + 1) * 512], psq[t4],
                             t4 % 2 == 1)
                    cast(kt[:, 1024:1536], psk[2], False)
                    cast(kt[:, 1536:2048], psk[3], True)

            # ---- filler machinery -------------------------------------
            # Projection strips (and their DMAs / v-transposes) are chopped
            # into ~0.85us units, pumped between attention emissions.
            class Filler:
                def __init__(self):
                    self.gens = []
                    self.emitted = 0

                def add(self, gen):
                    self.gens.append(gen)

                def pump(self, n):
                    done = 0
                    while done < n and self.gens:
                        try:
                            next(self.gens[0])
                            done += 1
                            self.emitted += 1
                        except StopIteration:
                            self.gens.pop(0)
                    return done

                def pump_to(self, total):
                    if total > self.emitted:
                        self.pump(total - self.emitted)

                def drain(self):
                    self.pump(1 << 30)

            filler = Filler()

            def dma_unit(src, idx, key):
                dma_w(src, idx, key)
                yield

            def proj_units(key, dest, dbase, t4s=(0, 1, 2, 3)):
                """[D, T] projection strip as filler units: 8 matmul units
                + 1 cast unit per 512-wide Tq block."""
                w = wts[key]
                for t4 in t4s:
                    ps = ps_tile("proj", f"ps_{key}_{t4}", bufs=2)[:]
                    for g in range(8):
                        for kc in range(g * 4, (g + 1) * 4):
                            nc.tensor.matmul(
                                ps,
                                lhsT=w[:, kc * 128:(kc + 1) * 128],
                                rhs=xs(kc, t4 * 512, 512),
                                start=(kc == 0), stop=(kc == KC - 1),
                            )
                        yield
                    nc.vector.tensor_copy(
                        out=dest[:, dbase + t4 * 512: dbase + (t4 + 1) * 512],
                        in_=ps)
                    yield

            def vtr_units(kv, vts, groups=(0, 1, 2, 3)):
                """PE-transpose the [D,T] v strip into vt [Tk,D] tiles,
                one 4-tile group (one Tq block) per unit."""
                for g in groups:
                    for m in range(g * 4, (g + 1) * 4):
                        ps = ps_tile("proj", f"ps_vt_{kv}_{m}",
                                     shape=(128, 128), dtyp=bf16, bufs=2)
                        nc.tensor.transpose(
                            ps[:], vts[:, m * 128:(m + 1) * 128], ident)
                        nc.vector.tensor_copy(
                            out=vt[:, (kv * NT + m) * VROW:
                                   (kv * NT + m) * VROW + D],
                            in_=ps[:])
                    yield

            def proj_v_units(kv, vts, t4s=(0, 1, 2, 3)):
                """v strip with its transposes interleaved: [9 strip units,
                1 vtr unit] per t4, so vt tiles for Tq block X are ready
                10*(X+1) units in."""
                pg = proj_units(f"v{kv}", vts, 0, t4s)
                vg = vtr_units(kv, vts, t4s)
                for _ in t4s:
                    for _ in range(9):
                        next(pg)
                        yield
                    next(vg)
                    yield

            # ---- attention --------------------------------------------
            last_block = [None]  # (h, b) of the final block, for tail split

            def attn_block(h, b):
                """One [512 Tq] block of head h; yields at filler points."""
                kv = h % 2
                qblk = qt[:, h * T + b * 512: h * T + (b + 1) * 512]
                # pv outs are VROW=129 fp32: pack two per PSUM bank
                # (offsets 0/256) so a block holds 2 of the 4 "pv" bufs
                # (double-buffered across blocks). The first PV matmul of
                # each group (j=4b, executed first) carries start=True.
                pvt = [ps_tile("pv", f"pv_{h}_{b}_{i}", bufs=4)
                       for i in range(2)]

                def pv_ap(s, lo, hi):
                    base = 256 * (s % 2)
                    return pvt[s // 2][:, base + lo: base + hi]

                # diag tiles first: their longer exp->mask->PV chain
                # overlaps filler instead of forming the block's tail.
                j_list = list(range(4 * b, 4 * b + 4)) + list(range(4 * b))
                first = True
                prev = None  # deferred (j, pt) whose PV is pending
                for j in j_list:
                    r = j - 4 * b  # >=0 on diagonal tiles
                    roff = max(0, r) * 128
                    spp = ps_tile("spp", f"sp_{h}_{b}_{j}", bufs=2)
                    nc.tensor.matmul(
                        spp[:, roff:512],
                        lhsT=kt[:, kv * T + j * 128: kv * T + (j + 1) * 128],
                        rhs=qblk[:, roff:512],
                        start=True, stop=True,
                    )
                    # alternate tags -> consecutive j land in non-adjacent
                    # SBUF slots, preventing the backend from fusing
                    # consecutive exps into one wide ACTIVATE.
                    pt = ptpool.tile([128, 512], bf16, name=f"pt_{h}_{b}_{j}",
                                     tag=f"pt{j % 2}")
                    nc.scalar.activation(pt[:, roff:512], spp[:, roff:512],
                                         EXP, scale=SCALE)
                    if r >= 0:
                        # true-diagonal 128-wide tile needs masking; PV for
                        # s > r proceeds straight after exp.
                        nc.vector.tensor_mul(
                            pt[:, roff: roff + 128],
                            pt[:, roff: roff + 128], tri)
                    yield
                    if first:
                        first = False
                        # zero the packed pv banks via PE (zeros
                        # stationary): start=True clears the FULL bank on
                        # TRN2 (measured), so packed groups must zero via
                        # matmul (order-independent) and pure-accumulate.
                        for i in range(2):
                            for base in (0, 256):
                                nc.tensor.matmul(
                                    pvt[i][:, base: base + VROW],
                                    lhsT=zer[:], rhs=qblk[:, 0:VROW],
                                    start=True, stop=False,
                                    skip_group_check=True)
                        yield
                    tail = (h, b) == last_block[0]
                    if prev is not None:
                        emit_pv(h, b, prev[0], prev[1], pv_ap)
                        if tail:
                            # terminal block (b=0): group s finishes at
                            # j=s, so normalize+DMA each group as soon as
                            # its accumulation closes, shortening the
                            # kernel tail to just the last group's chain.
                            norm_group(h, b, prev[0], pv_ap, prev[0] % 2)
                        yield
                    prev = (j, pt)
                emit_pv(h, b, prev[0], prev[1], pv_ap)
                if tail:
                    norm_group(h, b, prev[0], pv_ap, 1)
                    yield
                else:
                    yield
                    # normalize into one [128, 512] staging tile, single
                    # DMA for the whole block (4 separate 64KB DMAs
                    # serialize on the queue and stretch the tail). Both
                    # APs keep the partition dim leading.
                    ot = opool.tile([128, 512], f32, name=f"ot_{h}_{b}",
                                    tag="ot")
                    for s in range(4):
                        rec = recpool.tile([128, 1], f32,
                                           name=f"rec_{h}_{b}_{s}", tag="rec")
                        nc.vector.reciprocal(rec[:], pv_ap(s, D, D + 1))
                        nc.vector.tensor_scalar_mul(
                            ot[:, s * 128:(s + 1) * 128], pv_ap(s, 0, D),
                            rec[:])
                    nc.sync.dma_start(
                        out=out_d[b * 512:(b + 1) * 512, h * D:(h + 1) * D]
                        .rearrange("(s p) d -> p s d", s=4),
                        in_=ot[:].rearrange("p (s d) -> p s d", s=4))
                    yield

            def norm_group(h, b, s, pv_ap, on_act):
                rec = recpool.tile([128, 1], f32,
                                   name=f"rec_{h}_{b}_{s}", tag="rec")
                nc.vector.reciprocal(rec[:], pv_ap(s, D, D + 1))
                ot = opool.tile([128, 128], f32, name=f"otg_{h}_{b}_{s}",
                                tag="otg")
                if on_act:
                    nc.scalar.activation(ot[:], pv_ap(s, 0, D), COPY,
                                         scale=rec[:])
                else:
                    nc.vector.tensor_scalar_mul(ot[:], pv_ap(s, 0, D), rec[:])
                nc.sync.dma_start(
                    out=out_d[b * 512 + s * 128: b * 512 + (s + 1) * 128,
                              h * D:(h + 1) * D],
                    in_=ot[:])

            def emit_pv(h, b, j, pt, pv_ap):
                r = j - 4 * b
                kv = h % 2
                vsl = vt[:, (kv * NT + j) * VROW: (kv * NT + j + 1) * VROW]
                # exec order: diag js (4b..4b+3) first, then off-diag
                # 0..4b-1 -> stop on the last executed contribution.
                for s in range(max(0, r), 4):
                    nc.tensor.matmul(
                        pv_ap(s, 0, VROW),
                        lhsT=pt[:, s * 128: (s + 1) * 128],
                        rhs=vsl,
                        start=False,
                        stop=(j == ((4 * b - 1) if b > 0 else s)),
                        skip_group_check=True,
                    )

            # ---- DMA schedule -----------------------------------------
            # wq0/wk0 in pieces interleaved ahead of the x chunks they
            # gate; the first pieces are a single kc (64KB) so the first
            # matmul's gate (wq kc0 + x chunk 0) clears as early as
            # possible. masks deferred (first needed by vtr0, ~30us after
            # the x load completes).
            wq0 = wpool.tile([128, C], bf16, name="w_q0", tag="w")
            wk0 = wpool.tile([128, C], bf16, name="w_k0", tag="w")
            wts["q0"] = wq0
            wts["k0"] = wk0

            def dma_x_split(kc):
                xt = xpool.tile([128, T], bf16, name=f"xt{kc}", tag=f"xt{kc}")
                for t4 in range(4):
                    nc.sync.dma_start(
                        out=xt[:, t4 * 512:(t4 + 1) * 512],
                        in_=xT_d[:, kc * T + t4 * 512: kc * T + (t4 + 1) * 512])
                xts[kc] = xt

            nc.sync.dma_start(out=wq0[:, 0:128], in_=wq_d[0][:, 0:128])
            nc.sync.dma_start(out=wk0[:, 0:128], in_=wk_d[0][:, 0:128])
            dma_x_split(0)
            nc.sync.dma_start(out=wq0[:, 128:512], in_=wq_d[0][:, 128:512])
            nc.sync.dma_start(out=wk0[:, 128:512], in_=wk_d[0][:, 128:512])
            dma_x_split(1)
            dma_x_split(2)
            dma_x_split(3)
            for piece in range(1, 8):
                sl = slice(piece * 512, (piece + 1) * 512)
                nc.sync.dma_start(out=wq0[:, sl], in_=wq_d[0][:, sl])
                dma_x(piece * 4)
                nc.sync.dma_start(out=wk0[:, sl], in_=wk_d[0][:, sl])
                for kc in range(piece * 4 + 1, (piece + 1) * 4):
                    dma_x(kc)
            dma_w(wv_d, 0, "v0")
            nc.sync.dma_start(out=mask_sb[:], in_=mask_d[:])

            # ---- emission ---------------------------------------------
            # HAM warm-up: the PE is data-starved until ~10us (DMA queue
            # bring-up + first x chunk) and its clock gate opens slowly;
            # a short burst of dummy matmuls in the dead window opens the
            # gate before real work arrives, without delaying it.
            wps = ps_tile("pv", "warm", bufs=4)
            for _ in range(20):
                nc.tensor.matmul(wps[:, 0:128], lhsT=zer[:], rhs=zer[:],
                                 start=True, stop=True)
            proj_phase_a()

            # v0 head: t4=0 strip + its transposes run pure (nothing to
            # overlap yet); the rest becomes filler for attn0.
            vts0 = wpool.tile([128, T], bf16, name="vts_0", tag="w")
            for _ in proj_v_units(0, vts0, t4s=(0,)):
                pass

            # global filler queue: remaining v0 + all later strips.
            def chain_rest():
                yield from proj_v_units(0, vts0, t4s=(1, 2, 3))
                yield from dma_unit(wq_d, 2, "q2")
                yield from proj_units("q2", qt, 2 * T)
                yield from dma_unit(wq_d, 1, "q1")
                yield from proj_units("q1", qt, T)
                yield from dma_unit(wk_d, 1, "k1")
                yield from proj_units("k1", kt, T)
                yield from dma_unit(wv_d, 1, "v1")
                yield from proj_v_units(
                    1, wpool.tile([128, T], bf16, name="vts_1", tag="w"))
                yield from dma_unit(wq_d, 3, "q3")
                # t4 order (1,2,3,0): head 3 runs blocks [1,2,3,0], so the
                # final sub-strip (t4=0) feeds the final block and its units
                # stream through the second-to-last block's yields instead
                # of leaving the endgame with no filler.
                yield from proj_units("q3", qt, 3 * T, t4s=(1, 2, 3, 0))

            filler.add(chain_rest())

            # forced-drain unit counts (cumulative filler indices).
            # v0 rest: t4=1..3 interleaved with vtr0 groups, 10 units per
            # t4; vtr0 group b done at unit 10*b (b>=1).
            U_V0 = 30
            U_Q2 = [U_V0 + 1 + 9 * (t + 1) for t in range(4)]
            U_Q1 = U_Q2[3] + 1 + 36
            U_K1 = [U_Q1 + 1 + 9 * (t + 1) for t in range(4)]
            U_V1 = [U_K1[3] + 1 + 10 * (t + 1) for t in range(4)]
            base3 = U_V1[3] + 1
            U_Q3 = [base3 + 36, base3 + 9, base3 + 18, base3 + 27]
            N_UNITS = U_Q3[0]

            # attention blocks in dependency order, with forced drains;
            # background pump keeps filler flowing evenly (ratio of units
            # per yield).
            blocks = []
            for b in range(4):
                blocks.append((0, b, 10 * b))   # a0bX needs vtr0 group X
            for b in range(4):
                blocks.append((2, b, U_Q2[b]))
            for b in range(4):
                blocks.append((1, b, max(U_K1[b], U_V1[b])))
            # head 3 ends the kernel with its smallest block (b=0, whose
            # strip data drained long before): ~2us of exp vs ~8us for a
            # b=3 block, minimizing the ACT-over-PE exposure at the tail.
            for b in (1, 2, 3, 0):
                blocks.append((3, b, U_Q3[b]))
            last_block[0] = (3, 0)

            # stream filler gradually: during block i's yields, pump
            # linearly from its own need toward the next block's need, so
            # forced drains never burst and the endgame blocks keep filler.
            needs = [need for _, _, need in blocks] + [N_UNITS]
            for i, (h, b, need) in enumerate(blocks):
                filler.pump_to(need)
                nxt = max(needs[i + 1], need)
                ny = 2 * (4 * b + 4) + 2
                k = 0
                for _ in attn_block(h, b):
                    k += 1
                    filler.pump_to(need + (nxt - need) * k // ny)
            filler.drain()

    nc.compile()
    _prog_cache["nc"] = nc
    return nc


def _host_prep(x, Wq, bq, Wk, bk, Wv, bv):
    """Shard + repack inputs for the 8 cores. Returns in_maps list."""
    assert x.shape == (1, T, C)
    assert np.abs(bq).max() == 0 and np.abs(bk).max() == 0, \
        "nonzero q/k biases not supported"

    x0 = np.ascontiguousarray(x[0]).astype(BF16)
    # xT packed: [128, kc*T + t] = x[t, 128*kc + p]
    xT = np.ascontiguousarray(
        x0.reshape(T, KC, 128).transpose(2, 1, 0).reshape(128, KC * T))

    # causal diag-tile mask tri[tk, tq] = tq >= tk, plus identity
    tq = np.arange(128)[None, :]
    tk = np.arange(128)[:, None]
    masks = np.concatenate(
        [(tq >= tk).astype(BF16), np.eye(128, dtype=BF16)], axis=1)
    masks = np.ascontiguousarray(masks)

    def pack_w(Wrows):
        # Wrows: [128 (out c), C (in)] for one head ->
        # packed[p, 128*kc + c] = Wrows[c, 128*kc + p]
        return np.ascontiguousarray(
            Wrows.astype(BF16).reshape(128, KC, 128).transpose(2, 1, 0)
            .reshape(128, C))

    in_maps = []
    for c in range(N_CORES):
        qheads = [2 * c, 2 * c + 1, 2 * c + 16, 2 * c + 17]
        kvheads = [2 * c, 2 * c + 1]
        wq = np.stack([pack_w(Wq[128 * H:128 * (H + 1)]) for H in qheads])
        wk = np.stack([pack_w(Wk[128 * K:128 * (K + 1)]) for K in kvheads])
        wv = np.stack([pack_w(Wv[128 * K:128 * (K + 1)]) for K in kvheads])
        in_maps.append({
            "xT": xT, "wq": wq, "wk": wk, "wv": wv, "masks": masks,
        })
    return in_maps


def _assemble(results, bv):
    out = np.empty((T, C), dtype=np.float32)
    for c in range(N_CORES):
        r = results[c]["out"]
        qheads = [2 * c, 2 * c + 1, 2 * c + 16, 2 * c + 17]
        for i, H in enumerate(qheads):
            blk = r[:, 128 * i:128 * (i + 1)]
            if bv is not None:
                blk = blk + bv[128 * (H % N_KV_IDX):128 * (H % N_KV_IDX) + 128]
            out[:, 128 * H:128 * (H + 1)] = blk
    return out.reshape(1, T, C)


N_KV_IDX = 16


def _install_trace_hooks():
    """The agent image's antenv lacks axon_hooks; recreate it so
    run_bass_kernel_spmd's trace=True path can capture NTFF profiles."""
    import sys
    import types
    import antenv
    if "antenv.axon_hooks" not in sys.modules:
        mod = types.ModuleType("antenv.axon_hooks")
        mod._hook = None

        def set_axon_ntff_profile_hook(h):
            mod._hook = h

        def get_axon_ntff_profile_hook():
            return mod._hook

        mod.set_axon_ntff_profile_hook = set_axon_ntff_profile_hook
        mod.get_axon_ntff_profile_hook = get_axon_ntff_profile_hook
        sys.modules["antenv.axon_hooks"] = mod
        antenv.axon_hooks = mod
    from antenv.axon_hooks import (get_axon_ntff_profile_hook,
                                   set_axon_ntff_profile_hook)
    if get_axon_ntff_profile_hook() is None:
        if "/root/.axon_site" not in sys.path:
            sys.path.insert(0, "/root/.axon_site")
        from trn_agent_boot.trn_boot import _ntff_profile_via_ctypes
        set_axon_ntff_profile_hook(
            _ntff_profile_via_ctypes("/opt/axon/libaxon_pjrt.so"))
    import concourse.bass_utils as bu
    bu.upload_artifacts = lambda tmpdir: tmpdir


def _run(inputs, trace=False, trace_kwargs=None):
    if trace:
        _install_trace_hooks()
    from concourse.bass_utils import run_bass_kernel_spmd
    nc = _build_program()
    in_maps = _host_prep(**inputs)
    res = run_bass_kernel_spmd(
        nc, in_maps, list(range(N_CORES)), trace=trace,
        **(trace_kwargs or {}))
    bv = inputs["bv"].astype(np.float32)
    bv = bv if np.abs(bv).max() > 0 else None
    out = _assemble(res.results, bv)
    return out, res


def kernel(x, Wq, bq, Wk, bk, Wv, bv):
    out, _ = _run(dict(x=np.asarray(x), Wq=np.asarray(Wq), bq=np.asarray(bq),
                       Wk=np.asarray(Wk), bk=np.asarray(bk),
                       Wv=np.asarray(Wv), bv=np.asarray(bv)))
    return out


# revision 17
# speedup vs baseline: 1.0164x; 1.0164x over previous
"""GQA kernel for Trainium2, 8 NeuronCores, tensor-parallel over heads.

Problem: B=1, T=2048, C=4096, 32 q-heads, 16 kv-heads, head_dim=128,
scale = 1/sqrt(32), causal. q head H uses kv head H%16.

Sharding (no collectives needed): core c owns q-heads
{2c, 2c+1, 2c+16, 2c+17} and kv-heads {2c, 2c+1}. Each output column
block depends only on its own head, so the full output is a host-side
concat of per-core column slices.

Per-core kernel (all matmuls bf16, fp32 PSUM accumulation):
  xT resident in SBUF as [C=4096 (32 chunks of 128 part), T=2048].
  qT[h] = Wq_h @ xT    -> [128 (D), 2048 (T)]   (4 heads)
  kT[kv] = Wk_kv @ xT  -> [128 (D), 2048 (T)]   (2 kv heads)
  v[kv]  = x @ Wv_kv^T -> [2048 (T part), 128+1] (ones col for row sums)
  S^T tile (per 128-wide Tk tile j) = kT_j^T @ qT_block -> [128,512] PSUM
  P^T = exp(SCALE * S^T) (ACT), causal mask via {0,1} multiply on the
        true-diagonal 128-wide tile only; diagonal tiles narrowed to
        the live Tq columns.
  out[Tq,128] (+ row-sum col) = sum_j P^T_j.T @ v_j  (PSUM accum,
        4 Tq-chunks of 128 packed 2-per-bank, zeroed by a PE matmul)
  out normalized by reciprocal(row sum) (DVE), DMA'd out fp32.

Schedule (the big idea vs the 335us version): attention is paced by
the Scalar/ACT engine's exp (~21.6us per head vs ~19us of PE work),
and projections are paced by the PE with ACT idle. Interleaving them
hides the exp entirely: all post-phase-A projection strips are chopped
into 4-kc (0.85us) filler units and pumped into the PE queue between
each attention j-tile's S and PV, via one global filler queue with
forced drains at dependency points (block (h,b) forces its qt/kt/vt
producers). The 150-matmul HAM warmup is gone: phase A's real matmuls
start at first-DMA-arrival and absorb the clock ramp.
"""

import numpy as np
import ml_dtypes

BF16 = ml_dtypes.bfloat16
T = 2048
C = 4096
D = 128
N_HEADS = 32
N_KV = 16
SCALE = float(1.0 / np.sqrt(np.float32(N_HEADS)))
KC = C // 128          # 32 contraction chunks
NQH = 4                # local q heads per core
NKV = 2                # local kv heads per core
NT = T // 128          # 16 token tiles
VROW = D + 1           # 129: v with ones column
N_CORES = 8

_prog_cache = {}


def _build_program():
    if "nc" in _prog_cache:
        return _prog_cache["nc"]
    import concourse.bass as bass
    import concourse.tile as tile
    from concourse import bacc, mybir

    dt = mybir.dt
    f32 = dt.float32
    bf16 = dt.bfloat16
    EXP = mybir.ActivationFunctionType.Exp
    COPY = mybir.ActivationFunctionType.Copy

    nc = bacc.Bacc("TRN2", target_bir_lowering=False, debug=False,
                   num_devices=N_CORES)

    xT_d = nc.dram_tensor("xT", [128, KC * T], bf16, kind="ExternalInput").ap()
    wq_d = nc.dram_tensor("wq", [NQH, 128, C], bf16, kind="ExternalInput").ap()
    wk_d = nc.dram_tensor("wk", [NKV, 128, C], bf16, kind="ExternalInput").ap()
    wv_d = nc.dram_tensor("wv", [NKV, 128, C], bf16, kind="ExternalInput").ap()
    # masks: [128,128] causal diag tile + [128,128] identity for PE transpose
    mask_d = nc.dram_tensor("masks", [128, 256], bf16,
                            kind="ExternalInput").ap()
    out_d = nc.dram_tensor("out", [T, NQH * D], f32, kind="ExternalOutput").ap()

    with tile.TileContext(nc) as tc:
        with (
            tc.tile_pool(name="persist", bufs=1) as persist,
            tc.tile_pool(name="xpool", bufs=1) as xpool,
            tc.tile_pool(name="wpool", bufs=3) as wpool,
            tc.tile_pool(name="ptpool", bufs=3) as ptpool,
            tc.tile_pool(name="opool", bufs=4) as opool,
            tc.tile_pool(name="recpool", bufs=4) as recpool,
            # PSUM: 8 banks = pv 4x[128,512] + spp 2x[128,512] + proj 2x[128,512]
            tc.tile_pool(name="psum", bufs=1, space=bass.MemorySpace.PSUM) as psum,
        ):
            mask_sb = persist.tile([128, 256], bf16, name="mask_sb",
                                   tag="mask_sb")
            tri = mask_sb[:, 0:128]
            ident = mask_sb[:, 128:256]

            qt = persist.tile([128, NQH * T], bf16, name="qt", tag="qt")
            kt = persist.tile([128, NKV * T], bf16, name="kt", tag="kt")
            vt = persist.tile([128, NKV * NT * VROW], bf16, name="vt", tag="vt")
            zer = persist.tile([128, 128], bf16, name="zer", tag="zer")
            nc.vector.memset(zer[:], 0.0)

            # ones columns of v (row-sum trick)
            for i in range(NKV * NT):
                nc.vector.memset(vt[:, i * VROW + D: (i + 1) * VROW], 1.0)

            # xT is DMA'd as 32 per-kc chunks [128, T] so the startup
            # projections can consume chunks at DMA arrival pace.
            xts = [None] * KC
            wts = {}

            def dma_w(src, idx, key):
                w = wpool.tile([128, C], bf16, name=f"w_{key}", tag="w")
                nc.sync.dma_start(out=w[:], in_=src[idx])
                wts[key] = w

            def dma_x(kc):
                xt = xpool.tile([128, T], bf16, name=f"xt{kc}", tag=f"xt{kc}")
                nc.sync.dma_start(out=xt[:], in_=xT_d[:, kc * T:(kc + 1) * T])
                xts[kc] = xt

            def xs(kc, lo, size):
                return xts[kc][:, lo: lo + size]

            def ps_tile(tag, name, shape=(128, 512), dtyp=f32, bufs=None):
                return psum.tile(list(shape), dtyp, name=name, tag=tag,
                                 bufs=bufs)

            def proj_phase_a():
                """q0+k0 strips, kc-outer so program order matches x-chunk
                DMA arrival: PE does 8 matmuls (4.1K cycles) per 0.5MB chunk
                (~1.2us DMA), staying busy through the whole x load. Runs
                straight out of reset: the first couple kc's absorb the HAM
                clock ramp while the DMA stream is still the pacer."""
                wq = wts.pop("q0")
                wk = wts.pop("k0")
                psq = [ps_tile("pv", f"psA_q{t}", bufs=4)[:] for t in range(4)]
                psk = ([ps_tile("proj", f"psA_k{t}", bufs=2)[:]
                        for t in range(2)]
                       + [ps_tile("spp", f"psA_k{t + 2}", bufs=2)[:]
                          for t in range(2)])
                with nc.named_scope("phaseA"):
                    for kc in range(KC):
                        for t4 in range(4):
                            nc.tensor.matmul(
                                psq[t4], lhsT=wq[:, kc * 128:(kc + 1) * 128],
                                rhs=xs(kc, t4 * 512, 512),
                                start=(kc == 0), stop=(kc == KC - 1))
                        for t4 in range(4):
                            nc.tensor.matmul(
                                psk[t4], lhsT=wk[:, kc * 128:(kc + 1) * 128],
                                rhs=xs(kc, t4 * 512, 512),
                                start=(kc == 0), stop=(kc == KC - 1))
                    # cast order: k t4=0,1 first (frees the proj bufs for the
                    # v0 strip), then q (unblocks attn0's S), then k t4=2,3
                    # (frees the spp bufs). Alternate DVE/ACT to halve the
                    # chain.
                    def cast(dst, src, on_act):
                        if on_act:
                            nc.scalar.copy(out=dst, in_=src)
                        else:
                            nc.vector.tensor_copy(out=dst, in_=src)

                    cast(kt[:, 0:512], psk[0], False)
                    cast(kt[:, 512:1024], psk[1], True)
                    for t4 in range(4):
                        cast(qt[:, t4 * 512:(t4 + 1) * 512], psq[t4],
                             t4 % 2 == 1)
                    cast(kt[:, 1024:1536], psk[2], False)
                    cast(kt[:, 1536:2048], psk[3], True)

            # ---- filler machinery -------------------------------------
            # Projection strips (and their DMAs / v-transposes) are chopped
            # into ~0.85us units, pumped between attention emissions.
            class Filler:
                def __init__(self):
                    self.gens = []
                    self.emitted = 0

                def add(self, gen):
                    self.gens.append(gen)

                def pump(self, n):
                    done = 0
                    while done < n and self.gens:
                        try:
                            next(self.gens[0])
                            done += 1
                            self.emitted += 1
                        except StopIteration:
                            self.gens.pop(0)
                    return done

                def pump_to(self, total):
                    if total > self.emitted:
                        self.pump(total - self.emitted)

                def drain(self):
                    self.pump(1 << 30)

            filler = Filler()

            def dma_unit(src, idx, key):
                dma_w(src, idx, key)
                yield

            def proj_units(key, dest, dbase, t4s=(0, 1, 2, 3)):
                """[D, T] projection strip as filler units: 8 matmul units
                + 1 cast unit per 512-wide Tq block."""
                w = wts[key]
                for t4 in t4s:
                    ps = ps_tile("proj", f"ps_{key}_{t4}", bufs=2)[:]
                    for g in range(8):
                        for kc in range(g * 4, (g + 1) * 4):
                            nc.tensor.matmul(
                                ps,
                                lhsT=w[:, kc * 128:(kc + 1) * 128],
                                rhs=xs(kc, t4 * 512, 512),
                                start=(kc == 0), stop=(kc == KC - 1),
                            )
                        yield
                    nc.vector.tensor_copy(
                        out=dest[:, dbase + t4 * 512: dbase + (t4 + 1) * 512],
                        in_=ps)
                    yield

            def vtr_units(kv, vts, groups=(0, 1, 2, 3)):
                """PE-transpose the [D,T] v strip into vt [Tk,D] tiles,
                one 4-tile group (one Tq block) per unit."""
                for g in groups:
                    for m in range(g * 4, (g + 1) * 4):
                        ps = ps_tile("proj", f"ps_vt_{kv}_{m}",
                                     shape=(128, 128), dtyp=bf16, bufs=2)
                        nc.tensor.transpose(
                            ps[:], vts[:, m * 128:(m + 1) * 128], ident)
                        nc.vector.tensor_copy(
                            out=vt[:, (kv * NT + m) * VROW:
                                   (kv * NT + m) * VROW + D],
                            in_=ps[:])
                    yield

            def proj_v_units(kv, vts, t4s=(0, 1, 2, 3)):
                """v strip with its transposes interleaved: [9 strip units,
                1 vtr unit] per t4, so vt tiles for Tq block X are ready
                10*(X+1) units in."""
                pg = proj_units(f"v{kv}", vts, 0, t4s)
                vg = vtr_units(kv, vts, t4s)
                for _ in t4s:
                    for _ in range(9):
                        next(pg)
                        yield
                    next(vg)
                    yield

            # ---- attention --------------------------------------------
            last_block = [None]  # (h, b) of the final block, for tail split

            def attn_block(h, b):
                """One [512 Tq] block of head h; yields at filler points."""
                kv = h % 2
                qblk = qt[:, h * T + b * 512: h * T + (b + 1) * 512]
                # pv outs are VROW=129 fp32: pack two per PSUM bank
                # (offsets 0/256) so a block holds 2 of the 4 "pv" bufs
                # (double-buffered across blocks). The first PV matmul of
                # each group (j=4b, executed first) carries start=True.
                pvt = [ps_tile("pv", f"pv_{h}_{b}_{i}", bufs=4)
                       for i in range(2)]

                def pv_ap(s, lo, hi):
                    base = 256 * (s % 2)
                    return pvt[s // 2][:, base + lo: base + hi]

                # diag tiles first: their longer exp->mask->PV chain
                # overlaps filler instead of forming the block's tail.
                j_list = list(range(4 * b, 4 * b + 4)) + list(range(4 * b))
                first = True
                prev = None  # deferred (j, pt) whose PV is pending
                for j in j_list:
                    r = j - 4 * b  # >=0 on diagonal tiles
                    roff = max(0, r) * 128
                    spp = ps_tile("spp", f"sp_{h}_{b}_{j}", bufs=2)
                    nc.tensor.matmul(
                        spp[:, roff:512],
                        lhsT=kt[:, kv * T + j * 128: kv * T + (j + 1) * 128],
                        rhs=qblk[:, roff:512],
                        start=True, stop=True,
                    )
                    # alternate tags -> consecutive j land in non-adjacent
                    # SBUF slots, preventing the backend from fusing
                    # consecutive exps into one wide ACTIVATE.
                    pt = ptpool.tile([128, 512], bf16, name=f"pt_{h}_{b}_{j}",
                                     tag=f"pt{j % 2}")
                    nc.scalar.activation(pt[:, roff:512], spp[:, roff:512],
                                         EXP, scale=SCALE)
                    if r >= 0:
                        # true-diagonal 128-wide tile needs masking; PV for
                        # s > r proceeds straight after exp.
                        nc.vector.tensor_mul(
                            pt[:, roff: roff + 128],
                            pt[:, roff: roff + 128], tri)
                    yield
                    if first:
                        first = False
                        # zero the packed pv banks via PE (zeros
                        # stationary): start=True clears the FULL bank on
                        # TRN2 (measured), so packed groups must zero via
                        # matmul (order-independent) and pure-accumulate.
                        for i in range(2):
                            for base in (0, 256):
                                nc.tensor.matmul(
                                    pvt[i][:, base: base + VROW],
                                    lhsT=zer[:], rhs=qblk[:, 0:VROW],
                                    start=True, stop=False,
                                    skip_group_check=True)
                        yield
                    tail = (h, b) == last_block[0]
                    if prev is not None:
                        emit_pv(h, b, prev[0], prev[1], pv_ap)
                        if tail:
                            # terminal block (b=0): group s finishes at
                            # j=s, so normalize+DMA each group as soon as
                            # its accumulation closes, shortening the
                            # kernel tail to just the last group's chain.
                            norm_group(h, b, prev[0], pv_ap, prev[0] % 2)
                        yield
                    prev = (j, pt)
                emit_pv(h, b, prev[0], prev[1], pv_ap)
                if tail:
                    norm_group(h, b, prev[0], pv_ap, 1)
                    yield
                else:
                    yield
                    # normalize into one [128, 512] staging tile, single
                    # DMA for the whole block (4 separate 64KB DMAs
                    # serialize on the queue and stretch the tail). Both
                    # APs keep the partition dim leading.
                    ot = opool.tile([128, 512], f32, name=f"ot_{h}_{b}",
                                    tag="ot")
                    for s in range(4):
                        rec = recpool.tile([128, 1], f32,
                                           name=f"rec_{h}_{b}_{s}", tag="rec")
                        nc.vector.reciprocal(rec[:], pv_ap(s, D, D + 1))
                        nc.vector.tensor_scalar_mul(
                            ot[:, s * 128:(s + 1) * 128], pv_ap(s, 0, D),
                            rec[:])
                    nc.sync.dma_start(
                        out=out_d[b * 512:(b + 1) * 512, h * D:(h + 1) * D]
                        .rearrange("(s p) d -> p s d", s=4),
                        in_=ot[:].rearrange("p (s d) -> p s d", s=4))
                    yield

            def norm_group(h, b, s, pv_ap, on_act):
                rec = recpool.tile([128, 1], f32,
                                   name=f"rec_{h}_{b}_{s}", tag="rec")
                nc.vector.reciprocal(rec[:], pv_ap(s, D, D + 1))
                ot = opool.tile([128, 128], f32, name=f"otg_{h}_{b}_{s}",
                                tag="otg")
                if on_act:
                    nc.scalar.activation(ot[:], pv_ap(s, 0, D), COPY,
                                         scale=rec[:])
                else:
                    nc.vector.tensor_scalar_mul(ot[:], pv_ap(s, 0, D), rec[:])
                nc.sync.dma_start(
                    out=out_d[b * 512 + s * 128: b * 512 + (s + 1) * 128,
                              h * D:(h + 1) * D],
                    in_=ot[:])

            def emit_pv(h, b, j, pt, pv_ap):
                r = j - 4 * b
                kv = h % 2
                vsl = vt[:, (kv * NT + j) * VROW: (kv * NT + j + 1) * VROW]
                # exec order: diag js (4b..4b+3) first, then off-diag
                # 0..4b-1 -> stop on the last executed contribution.
                for s in range(max(0, r), 4):
                    nc.tensor.matmul(
                        pv_ap(s, 0, VROW),
                        lhsT=pt[:, s * 128: (s + 1) * 128],
                        rhs=vsl,
                        start=False,
                        stop=(j == ((4 * b - 1) if b > 0 else s)),
                        skip_group_check=True,
                    )

            # ---- DMA schedule -----------------------------------------
            # wq0/wk0 in pieces interleaved ahead of the x chunks they
            # gate; the first pieces are a single kc (64KB) so the first
            # matmul's gate (wq kc0 + x chunk 0) clears as early as
            # possible. masks deferred (first needed by vtr0, ~30us after
            # the x load completes).
            wq0 = wpool.tile([128, C], bf16, name="w_q0", tag="w")
            wk0 = wpool.tile([128, C], bf16, name="w_k0", tag="w")
            wts["q0"] = wq0
            wts["k0"] = wk0

            nc.sync.dma_start(out=wq0[:, 0:128], in_=wq_d[0][:, 0:128])
            dma_x(0)
            nc.sync.dma_start(out=wk0[:, 0:128], in_=wk_d[0][:, 0:128])
            nc.sync.dma_start(out=wq0[:, 128:512], in_=wq_d[0][:, 128:512])
            dma_x(1)
            nc.sync.dma_start(out=wk0[:, 128:512], in_=wk_d[0][:, 128:512])
            dma_x(2)
            dma_x(3)
            for piece in range(1, 8):
                sl = slice(piece * 512, (piece + 1) * 512)
                nc.sync.dma_start(out=wq0[:, sl], in_=wq_d[0][:, sl])
                dma_x(piece * 4)
                nc.sync.dma_start(out=wk0[:, sl], in_=wk_d[0][:, sl])
                for kc in range(piece * 4 + 1, (piece + 1) * 4):
                    dma_x(kc)
            dma_w(wv_d, 0, "v0")
            nc.sync.dma_start(out=mask_sb[:], in_=mask_d[:])

            # ---- emission ---------------------------------------------
            # HAM warm-up: the PE is data-starved until ~10us (DMA queue
            # bring-up + first x chunk) and its clock gate opens slowly;
            # a short burst of dummy matmuls in the dead window opens the
            # gate before real work arrives, without delaying it.
            wps = ps_tile("pv", "warm", bufs=4)
            for _ in range(40):
                nc.tensor.matmul(wps[:, 0:128], lhsT=zer[:], rhs=zer[:],
                                 start=True, stop=True)
            proj_phase_a()

            # v0 head: t4=0 strip + its transposes run pure (nothing to
            # overlap yet); the rest becomes filler for attn0.
            vts0 = wpool.tile([128, T], bf16, name="vts_0", tag="w")
            for _ in proj_v_units(0, vts0, t4s=(0,)):
                pass

            # global filler queue: remaining v0 + all later strips.
            def chain_rest():
                yield from proj_v_units(0, vts0, t4s=(1, 2, 3))
                yield from dma_unit(wq_d, 2, "q2")
                yield from proj_units("q2", qt, 2 * T)
                yield from dma_unit(wq_d, 1, "q1")
                yield from proj_units("q1", qt, T)
                yield from dma_unit(wk_d, 1, "k1")
                yield from proj_units("k1", kt, T)
                yield from dma_unit(wv_d, 1, "v1")
                yield from proj_v_units(
                    1, wpool.tile([128, T], bf16, name="vts_1", tag="w"))
                yield from dma_unit(wq_d, 3, "q3")
                # t4 order (1,2,3,0): head 3 runs blocks [1,2,3,0], so the
                # final sub-strip (t4=0) feeds the final block and its units
                # stream through the second-to-last block's yields instead
                # of leaving the endgame with no filler.
                yield from proj_units("q3", qt, 3 * T, t4s=(1, 2, 3, 0))

            filler.add(chain_rest())

            # forced-drain unit counts (cumulative filler indices).
            # v0 rest: t4=1..3 interleaved with vtr0 groups, 10 units per
            # t4; vtr0 group b done at unit 10*b (b>=1).
            U_V0 = 30
            U_Q2 = [U_V0 + 1 + 9 * (t + 1) for t in range(4)]
            U_Q1 = U_Q2[3] + 1 + 36
            U_K1 = [U_Q1 + 1 + 9 * (t + 1) for t in range(4)]
            U_V1 = [U_K1[3] + 1 + 10 * (t + 1) for t in range(4)]
            base3 = U_V1[3] + 1
            U_Q3 = [base3 + 36, base3 + 9, base3 + 18, base3 + 27]
            N_UNITS = U_Q3[0]

            # attention blocks in dependency order, with forced drains;
            # background pump keeps filler flowing evenly (ratio of units
            # per yield).
            blocks = []
            for b in range(4):
                blocks.append((0, b, 10 * b))   # a0bX needs vtr0 group X
            for b in range(4):
                blocks.append((2, b, U_Q2[b]))
            for b in range(4):
                blocks.append((1, b, max(U_K1[b], U_V1[b])))
            # head 3 ends the kernel with its smallest block (b=0, whose
            # strip data drained long before): ~2us of exp vs ~8us for a
            # b=3 block, minimizing the ACT-over-PE exposure at the tail.
            for b in (1, 2, 3, 0):
                blocks.append((3, b, U_Q3[b]))
            last_block[0] = (3, 0)

            # stream filler gradually: during block i's yields, pump
            # linearly from its own need toward the next block's need, so
            # forced drains never burst and the endgame blocks keep filler.
            needs = [need for _, _, need in blocks] + [N_UNITS]
            for i, (h, b, need) in enumerate(blocks):
                filler.pump_to(need)
                nxt = max(needs[i + 1], need)
                ny = 2 * (4 * b + 4) + 2
                k = 0
                for _ in attn_block(h, b):
                    k += 1
                    filler.pump_to(need + (nxt - need) * k // ny)
            filler.drain()

    nc.compile()
    _prog_cache["nc"] = nc
    return nc


def _host_prep(x, Wq, bq, Wk, bk, Wv, bv):
    """Shard + repack inputs for the 8 cores. Returns in_maps list."""
    assert x.shape == (1, T, C)
    assert np.abs(bq).max() == 0 and np.abs(bk).max() == 0, \
        "nonzero q/k biases not supported"

    x0 = np.ascontiguousarray(x[0]).astype(BF16)
    # xT packed: [128, kc*T + t] = x[t, 128*kc + p]
    xT = np.ascontiguousarray(
        x0.reshape(T, KC, 128).transpose(2, 1, 0).reshape(128, KC * T))

    # causal diag-tile mask tri[tk, tq] = tq >= tk, plus identity
    tq = np.arange(128)[None, :]
    tk = np.arange(128)[:, None]
    masks = np.concatenate(
        [(tq >= tk).astype(BF16), np.eye(128, dtype=BF16)], axis=1)
    masks = np.ascontiguousarray(masks)

    def pack_w(Wrows):
        # Wrows: [128 (out c), C (in)] for one head ->
        # packed[p, 128*kc + c] = Wrows[c, 128*kc + p]
        return np.ascontiguousarray(
            Wrows.astype(BF16).reshape(128, KC, 128).transpose(2, 1, 0)
            .reshape(128, C))

    in_maps = []
    for c in range(N_CORES):
        qheads = [2 * c, 2 * c + 1, 2 * c + 16, 2 * c + 17]
        kvheads = [2 * c, 2 * c + 1]
        wq = np.stack([pack_w(Wq[128 * H:128 * (H + 1)]) for H in qheads])
        wk = np.stack([pack_w(Wk[128 * K:128 * (K + 1)]) for K in kvheads])
        wv = np.stack([pack_w(Wv[128 * K:128 * (K + 1)]) for K in kvheads])
        in_maps.append({
            "xT": xT, "wq": wq, "wk": wk, "wv": wv, "masks": masks,
        })
    return in_maps


def _assemble(results, bv):
    out = np.empty((T, C), dtype=np.float32)
    for c in range(N_CORES):
        r = results[c]["out"]
        qheads = [2 * c, 2 * c + 1, 2 * c + 16, 2 * c + 17]
        for i, H in enumerate(qheads):
            blk = r[:, 128 * i:128 * (i + 1)]
            if bv is not None:
                blk = blk + bv[128 * (H % N_KV_IDX):128 * (H % N_KV_IDX) + 128]
            out[:, 128 * H:128 * (H + 1)] = blk
    return out.reshape(1, T, C)


N_KV_IDX = 16


def _install_trace_hooks():
    """The agent image's antenv lacks axon_hooks; recreate it so
    run_bass_kernel_spmd's trace=True path can capture NTFF profiles."""
    import sys
    import types
    import antenv
    if "antenv.axon_hooks" not in sys.modules:
        mod = types.ModuleType("antenv.axon_hooks")
        mod._hook = None

        def set_axon_ntff_profile_hook(h):
            mod._hook = h

        def get_axon_ntff_profile_hook():
            return mod._hook

        mod.set_axon_ntff_profile_hook = set_axon_ntff_profile_hook
        mod.get_axon_ntff_profile_hook = get_axon_ntff_profile_hook
        sys.modules["antenv.axon_hooks"] = mod
        antenv.axon_hooks = mod
    from antenv.axon_hooks import (get_axon_ntff_profile_hook,
                                   set_axon_ntff_profile_hook)
    if get_axon_ntff_profile_hook() is None:
        if "/root/.axon_site" not in sys.path:
            sys.path.insert(0, "/root/.axon_site")
        from trn_agent_boot.trn_boot import _ntff_profile_via_ctypes
        set_axon_ntff_profile_hook(
            _ntff_profile_via_ctypes("/opt/axon/libaxon_pjrt.so"))
    import concourse.bass_utils as bu
    bu.upload_artifacts = lambda tmpdir: tmpdir


def _run(inputs, trace=False, trace_kwargs=None):
    if trace:
        _install_trace_hooks()
    from concourse.bass_utils import run_bass_kernel_spmd
    nc = _build_program()
    in_maps = _host_prep(**inputs)
    res = run_bass_kernel_spmd(
        nc, in_maps, list(range(N_CORES)), trace=trace,
        **(trace_kwargs or {}))
    bv = inputs["bv"].astype(np.float32)
    bv = bv if np.abs(bv).max() > 0 else None
    out = _assemble(res.results, bv)
    return out, res


def kernel(x, Wq, bq, Wk, bk, Wv, bv):
    out, _ = _run(dict(x=np.asarray(x), Wq=np.asarray(Wq), bq=np.asarray(bq),
                       Wk=np.asarray(Wk), bk=np.asarray(bk),
                       Wv=np.asarray(Wv), bv=np.asarray(bv)))
    return out


# revision 18
# speedup vs baseline: 1.0166x; 1.0002x over previous
"""GQA kernel for Trainium2, 8 NeuronCores, tensor-parallel over heads.

Problem: B=1, T=2048, C=4096, 32 q-heads, 16 kv-heads, head_dim=128,
scale = 1/sqrt(32), causal. q head H uses kv head H%16.

Sharding (no collectives needed): core c owns q-heads
{2c, 2c+1, 2c+16, 2c+17} and kv-heads {2c, 2c+1}. Each output column
block depends only on its own head, so the full output is a host-side
concat of per-core column slices.

Per-core kernel (all matmuls bf16, fp32 PSUM accumulation):
  xT resident in SBUF as [C=4096 (32 chunks of 128 part), T=2048].
  qT[h] = Wq_h @ xT    -> [128 (D), 2048 (T)]   (4 heads)
  kT[kv] = Wk_kv @ xT  -> [128 (D), 2048 (T)]   (2 kv heads)
  v[kv]  = x @ Wv_kv^T -> [2048 (T part), 128+1] (ones col for row sums)
  S^T tile (per 128-wide Tk tile j) = kT_j^T @ qT_block -> [128,512] PSUM
  P^T = exp(SCALE * S^T) (ACT), causal mask via {0,1} multiply on the
        true-diagonal 128-wide tile only; diagonal tiles narrowed to
        the live Tq columns.
  out[Tq,128] (+ row-sum col) = sum_j P^T_j.T @ v_j  (PSUM accum,
        4 Tq-chunks of 128 packed 2-per-bank, zeroed by a PE matmul)
  out normalized by reciprocal(row sum) (DVE), DMA'd out fp32.

Schedule (the big idea vs the 335us version): attention is paced by
the Scalar/ACT engine's exp (~21.6us per head vs ~19us of PE work),
and projections are paced by the PE with ACT idle. Interleaving them
hides the exp entirely: all post-phase-A projection strips are chopped
into 4-kc (0.85us) filler units and pumped into the PE queue between
each attention j-tile's S and PV, via one global filler queue with
forced drains at dependency points (block (h,b) forces its qt/kt/vt
producers). The 150-matmul HAM warmup is gone: phase A's real matmuls
start at first-DMA-arrival and absorb the clock ramp.
"""

import numpy as np
import ml_dtypes

BF16 = ml_dtypes.bfloat16
T = 2048
C = 4096
D = 128
N_HEADS = 32
N_KV = 16
SCALE = float(1.0 / np.sqrt(np.float32(N_HEADS)))
KC = C // 128          # 32 contraction chunks
NQH = 4                # local q heads per core
NKV = 2                # local kv heads per core
NT = T // 128          # 16 token tiles
VROW = D + 1           # 129: v with ones column
N_CORES = 8

_prog_cache = {}


def _build_program():
    if "nc" in _prog_cache:
        return _prog_cache["nc"]
    import concourse.bass as bass
    import concourse.tile as tile
    from concourse import bacc, mybir

    dt = mybir.dt
    f32 = dt.float32
    bf16 = dt.bfloat16
    EXP = mybir.ActivationFunctionType.Exp
    COPY = mybir.ActivationFunctionType.Copy

    nc = bacc.Bacc("TRN2", target_bir_lowering=False, debug=False,
                   num_devices=N_CORES)

    xT_d = nc.dram_tensor("xT", [KC, 128, T], bf16,
                          kind="ExternalInput").ap()
    wq_d = nc.dram_tensor("wq", [NQH, 128, C], bf16, kind="ExternalInput").ap()
    wk_d = nc.dram_tensor("wk", [NKV, 128, C], bf16, kind="ExternalInput").ap()
    wv_d = nc.dram_tensor("wv", [NKV, 128, C], bf16, kind="ExternalInput").ap()
    # masks: [128,128] causal diag tile + [128,128] identity for PE transpose
    mask_d = nc.dram_tensor("masks", [128, 256], bf16,
                            kind="ExternalInput").ap()
    out_d = nc.dram_tensor("out", [T, NQH * D], f32, kind="ExternalOutput").ap()

    with tile.TileContext(nc) as tc:
        with (
            tc.tile_pool(name="persist", bufs=1) as persist,
            tc.tile_pool(name="xpool", bufs=1) as xpool,
            tc.tile_pool(name="wpool", bufs=3) as wpool,
            tc.tile_pool(name="ptpool", bufs=3) as ptpool,
            tc.tile_pool(name="opool", bufs=4) as opool,
            tc.tile_pool(name="recpool", bufs=4) as recpool,
            # PSUM: 8 banks = pv 4x[128,512] + spp 2x[128,512] + proj 2x[128,512]
            tc.tile_pool(name="psum", bufs=1, space=bass.MemorySpace.PSUM) as psum,
        ):
            mask_sb = persist.tile([128, 256], bf16, name="mask_sb",
                                   tag="mask_sb")
            tri = mask_sb[:, 0:128]
            ident = mask_sb[:, 128:256]

            qt = persist.tile([128, NQH * T], bf16, name="qt", tag="qt")
            kt = persist.tile([128, NKV * T], bf16, name="kt", tag="kt")
            vt = persist.tile([128, NKV * NT * VROW], bf16, name="vt", tag="vt")
            zer = persist.tile([128, 128], bf16, name="zer", tag="zer")
            nc.vector.memset(zer[:], 0.0)

            # ones columns of v (row-sum trick)
            for i in range(NKV * NT):
                nc.vector.memset(vt[:, i * VROW + D: (i + 1) * VROW], 1.0)

            # xT is DMA'd as 32 per-kc chunks [128, T] so the startup
            # projections can consume chunks at DMA arrival pace.
            xts = [None] * KC
            wts = {}

            def dma_w(src, idx, key):
                w = wpool.tile([128, C], bf16, name=f"w_{key}", tag="w")
                nc.sync.dma_start(out=w[:], in_=src[idx])
                wts[key] = w

            def dma_x(kc):
                xt = xpool.tile([128, T], bf16, name=f"xt{kc}", tag=f"xt{kc}")
                nc.sync.dma_start(out=xt[:], in_=xT_d[kc])
                xts[kc] = xt

            def xs(kc, lo, size):
                return xts[kc][:, lo: lo + size]

            def ps_tile(tag, name, shape=(128, 512), dtyp=f32, bufs=None):
                return psum.tile(list(shape), dtyp, name=name, tag=tag,
                                 bufs=bufs)

            def proj_phase_a():
                """q0+k0 strips, kc-outer so program order matches x-chunk
                DMA arrival: PE does 8 matmuls (4.1K cycles) per 0.5MB chunk
                (~1.2us DMA), staying busy through the whole x load. Runs
                straight out of reset: the first couple kc's absorb the HAM
                clock ramp while the DMA stream is still the pacer."""
                wq = wts.pop("q0")
                wk = wts.pop("k0")
                psq = [ps_tile("pv", f"psA_q{t}", bufs=4)[:] for t in range(4)]
                psk = ([ps_tile("proj", f"psA_k{t}", bufs=2)[:]
                        for t in range(2)]
                       + [ps_tile("spp", f"psA_k{t + 2}", bufs=2)[:]
                          for t in range(2)])
                with nc.named_scope("phaseA"):
                    for kc in range(KC):
                        for t4 in range(4):
                            nc.tensor.matmul(
                                psq[t4], lhsT=wq[:, kc * 128:(kc + 1) * 128],
                                rhs=xs(kc, t4 * 512, 512),
                                start=(kc == 0), stop=(kc == KC - 1))
                        for t4 in range(4):
                            nc.tensor.matmul(
                                psk[t4], lhsT=wk[:, kc * 128:(kc + 1) * 128],
                                rhs=xs(kc, t4 * 512, 512),
                                start=(kc == 0), stop=(kc == KC - 1))
                    # cast order: k t4=0,1 first (frees the proj bufs for the
                    # v0 strip), then q (unblocks attn0's S), then k t4=2,3
                    # (frees the spp bufs). Alternate DVE/ACT to halve the
                    # chain.
                    def cast(dst, src, on_act):
                        if on_act:
                            nc.scalar.copy(out=dst, in_=src)
                        else:
                            nc.vector.tensor_copy(out=dst, in_=src)

                    cast(kt[:, 0:512], psk[0], False)
                    cast(kt[:, 512:1024], psk[1], True)
                    for t4 in range(4):
                        cast(qt[:, t4 * 512:(t4 + 1) * 512], psq[t4],
                             t4 % 2 == 1)
                    cast(kt[:, 1024:1536], psk[2], False)
                    cast(kt[:, 1536:2048], psk[3], True)

            # ---- filler machinery -------------------------------------
            # Projection strips (and their DMAs / v-transposes) are chopped
            # into ~0.85us units, pumped between attention emissions.
            class Filler:
                def __init__(self):
                    self.gens = []
                    self.emitted = 0

                def add(self, gen):
                    self.gens.append(gen)

                def pump(self, n):
                    done = 0
                    while done < n and self.gens:
                        try:
                            next(self.gens[0])
                            done += 1
                            self.emitted += 1
                        except StopIteration:
                            self.gens.pop(0)
                    return done

                def pump_to(self, total):
                    if total > self.emitted:
                        self.pump(total - self.emitted)

                def drain(self):
                    self.pump(1 << 30)

            filler = Filler()

            def dma_unit(src, idx, key):
                dma_w(src, idx, key)
                yield

            def proj_units(key, dest, dbase, t4s=(0, 1, 2, 3)):
                """[D, T] projection strip as filler units: 8 matmul units
                + 1 cast unit per 512-wide Tq block."""
                w = wts[key]
                for t4 in t4s:
                    ps = ps_tile("proj", f"ps_{key}_{t4}", bufs=2)[:]
                    for g in range(8):
                        for kc in range(g * 4, (g + 1) * 4):
                            nc.tensor.matmul(
                                ps,
                                lhsT=w[:, kc * 128:(kc + 1) * 128],
                                rhs=xs(kc, t4 * 512, 512),
                                start=(kc == 0), stop=(kc == KC - 1),
                            )
                        yield
                    nc.vector.tensor_copy(
                        out=dest[:, dbase + t4 * 512: dbase + (t4 + 1) * 512],
                        in_=ps)
                    yield

            def vtr_units(kv, vts, groups=(0, 1, 2, 3)):
                """PE-transpose the [D,T] v strip into vt [Tk,D] tiles,
                one 4-tile group (one Tq block) per unit."""
                for g in groups:
                    for m in range(g * 4, (g + 1) * 4):
                        ps = ps_tile("proj", f"ps_vt_{kv}_{m}",
                                     shape=(128, 128), dtyp=bf16, bufs=2)
                        nc.tensor.transpose(
                            ps[:], vts[:, m * 128:(m + 1) * 128], ident)
                        nc.vector.tensor_copy(
                            out=vt[:, (kv * NT + m) * VROW:
                                   (kv * NT + m) * VROW + D],
                            in_=ps[:])
                    yield

            def proj_v_units(kv, vts, t4s=(0, 1, 2, 3)):
                """v strip with its transposes interleaved: [9 strip units,
                1 vtr unit] per t4, so vt tiles for Tq block X are ready
                10*(X+1) units in."""
                pg = proj_units(f"v{kv}", vts, 0, t4s)
                vg = vtr_units(kv, vts, t4s)
                for _ in t4s:
                    for _ in range(9):
                        next(pg)
                        yield
                    next(vg)
                    yield

            # ---- attention --------------------------------------------
            last_block = [None]  # (h, b) of the final block, for tail split

            def attn_block(h, b):
                """One [512 Tq] block of head h; yields at filler points."""
                kv = h % 2
                qblk = qt[:, h * T + b * 512: h * T + (b + 1) * 512]
                # pv outs are VROW=129 fp32: pack two per PSUM bank
                # (offsets 0/256) so a block holds 2 of the 4 "pv" bufs
                # (double-buffered across blocks). The first PV matmul of
                # each group (j=4b, executed first) carries start=True.
                pvt = [ps_tile("pv", f"pv_{h}_{b}_{i}", bufs=4)
                       for i in range(2)]

                def pv_ap(s, lo, hi):
                    base = 256 * (s % 2)
                    return pvt[s // 2][:, base + lo: base + hi]

                # diag tiles first: their longer exp->mask->PV chain
                # overlaps filler instead of forming the block's tail.
                j_list = list(range(4 * b, 4 * b + 4)) + list(range(4 * b))
                first = True
                prev = None  # deferred (j, pt) whose PV is pending
                for j in j_list:
                    r = j - 4 * b  # >=0 on diagonal tiles
                    roff = max(0, r) * 128
                    spp = ps_tile("spp", f"sp_{h}_{b}_{j}", bufs=2)
                    nc.tensor.matmul(
                        spp[:, roff:512],
                        lhsT=kt[:, kv * T + j * 128: kv * T + (j + 1) * 128],
                        rhs=qblk[:, roff:512],
                        start=True, stop=True,
                    )
                    # alternate tags -> consecutive j land in non-adjacent
                    # SBUF slots, preventing the backend from fusing
                    # consecutive exps into one wide ACTIVATE.
                    pt = ptpool.tile([128, 512], bf16, name=f"pt_{h}_{b}_{j}",
                                     tag=f"pt{j % 2}")
                    nc.scalar.activation(pt[:, roff:512], spp[:, roff:512],
                                         EXP, scale=SCALE)
                    if r >= 0:
                        # true-diagonal 128-wide tile needs masking; PV for
                        # s > r proceeds straight after exp.
                        nc.vector.tensor_mul(
                            pt[:, roff: roff + 128],
                            pt[:, roff: roff + 128], tri)
                    yield
                    if first:
                        first = False
                        # zero the packed pv banks via PE (zeros
                        # stationary): start=True clears the FULL bank on
                        # TRN2 (measured), so packed groups must zero via
                        # matmul (order-independent) and pure-accumulate.
                        for i in range(2):
                            for base in (0, 256):
                                nc.tensor.matmul(
                                    pvt[i][:, base: base + VROW],
                                    lhsT=zer[:], rhs=qblk[:, 0:VROW],
                                    start=True, stop=False,
                                    skip_group_check=True)
                        yield
                    tail = (h, b) == last_block[0]
                    if prev is not None:
                        emit_pv(h, b, prev[0], prev[1], pv_ap)
                        if tail:
                            # terminal block (b=0): group s finishes at
                            # j=s, so normalize+DMA each group as soon as
                            # its accumulation closes, shortening the
                            # kernel tail to just the last group's chain.
                            norm_group(h, b, prev[0], pv_ap, prev[0] % 2)
                        yield
                    prev = (j, pt)
                emit_pv(h, b, prev[0], prev[1], pv_ap)
                if tail:
                    norm_group(h, b, prev[0], pv_ap, 1)
                    yield
                else:
                    yield
                    # normalize into one [128, 512] staging tile, single
                    # DMA for the whole block (4 separate 64KB DMAs
                    # serialize on the queue and stretch the tail). Both
                    # APs keep the partition dim leading.
                    ot = opool.tile([128, 512], f32, name=f"ot_{h}_{b}",
                                    tag="ot")
                    for s in range(4):
                        rec = recpool.tile([128, 1], f32,
                                           name=f"rec_{h}_{b}_{s}", tag="rec")
                        nc.vector.reciprocal(rec[:], pv_ap(s, D, D + 1))
                        nc.vector.tensor_scalar_mul(
                            ot[:, s * 128:(s + 1) * 128], pv_ap(s, 0, D),
                            rec[:])
                    nc.sync.dma_start(
                        out=out_d[b * 512:(b + 1) * 512, h * D:(h + 1) * D]
                        .rearrange("(s p) d -> p s d", s=4),
                        in_=ot[:].rearrange("p (s d) -> p s d", s=4))
                    yield

            def norm_group(h, b, s, pv_ap, on_act):
                rec = recpool.tile([128, 1], f32,
                                   name=f"rec_{h}_{b}_{s}", tag="rec")
                nc.vector.reciprocal(rec[:], pv_ap(s, D, D + 1))
                ot = opool.tile([128, 128], f32, name=f"otg_{h}_{b}_{s}",
                                tag="otg")
                if on_act:
                    nc.scalar.activation(ot[:], pv_ap(s, 0, D), COPY,
                                         scale=rec[:])
                else:
                    nc.vector.tensor_scalar_mul(ot[:], pv_ap(s, 0, D), rec[:])
                nc.sync.dma_start(
                    out=out_d[b * 512 + s * 128: b * 512 + (s + 1) * 128,
                              h * D:(h + 1) * D],
                    in_=ot[:])

            def emit_pv(h, b, j, pt, pv_ap):
                r = j - 4 * b
                kv = h % 2
                vsl = vt[:, (kv * NT + j) * VROW: (kv * NT + j + 1) * VROW]
                # exec order: diag js (4b..4b+3) first, then off-diag
                # 0..4b-1 -> stop on the last executed contribution.
                for s in range(max(0, r), 4):
                    nc.tensor.matmul(
                        pv_ap(s, 0, VROW),
                        lhsT=pt[:, s * 128: (s + 1) * 128],
                        rhs=vsl,
                        start=False,
                        stop=(j == ((4 * b - 1) if b > 0 else s)),
                        skip_group_check=True,
                    )

            # ---- DMA schedule -----------------------------------------
            # wq0/wk0 in pieces interleaved ahead of the x chunks they
            # gate; the first pieces are a single kc (64KB) so the first
            # matmul's gate (wq kc0 + x chunk 0) clears as early as
            # possible. masks deferred (first needed by vtr0, ~30us after
            # the x load completes).
            wq0 = wpool.tile([128, C], bf16, name="w_q0", tag="w")
            wk0 = wpool.tile([128, C], bf16, name="w_k0", tag="w")
            wts["q0"] = wq0
            wts["k0"] = wk0

            nc.sync.dma_start(out=wq0[:, 0:128], in_=wq_d[0][:, 0:128])
            xt0 = xpool.tile([128, T], bf16, name="xt0", tag="xt0")
            xts[0] = xt0
            nc.sync.dma_start(out=xt0[:, 0:512], in_=xT_d[0][:, 0:512])
            nc.sync.dma_start(out=wk0[:, 0:128], in_=wk_d[0][:, 0:128])
            nc.sync.dma_start(out=xt0[:, 512:2048], in_=xT_d[0][:, 512:2048])
            nc.sync.dma_start(out=wq0[:, 128:512], in_=wq_d[0][:, 128:512])
            dma_x(1)
            nc.sync.dma_start(out=wk0[:, 128:512], in_=wk_d[0][:, 128:512])
            dma_x(2)
            dma_x(3)
            for piece in range(1, 8):
                sl = slice(piece * 512, (piece + 1) * 512)
                nc.sync.dma_start(out=wq0[:, sl], in_=wq_d[0][:, sl])
                dma_x(piece * 4)
                nc.sync.dma_start(out=wk0[:, sl], in_=wk_d[0][:, sl])
                for kc in range(piece * 4 + 1, (piece + 1) * 4):
                    dma_x(kc)
            dma_w(wv_d, 0, "v0")
            nc.sync.dma_start(out=mask_sb[:], in_=mask_d[:])

            # ---- emission ---------------------------------------------
            # HAM warm-up: the PE is data-starved until ~10us (DMA queue
            # bring-up + first x chunk) and its clock gate opens slowly;
            # a short burst of dummy matmuls in the dead window opens the
            # gate before real work arrives, without delaying it.
            wps = ps_tile("pv", "warm", bufs=4)
            for _ in range(32):
                nc.tensor.matmul(wps[:, 0:128], lhsT=zer[:], rhs=zer[:],
                                 start=True, stop=True)
            proj_phase_a()

            # v0 head: t4=0 strip + its transposes run pure (nothing to
            # overlap yet); the rest becomes filler for attn0.
            vts0 = wpool.tile([128, T], bf16, name="vts_0", tag="w")
            for _ in proj_v_units(0, vts0, t4s=(0,)):
                pass

            # global filler queue: remaining v0 + all later strips.
            def chain_rest():
                yield from proj_v_units(0, vts0, t4s=(1, 2, 3))
                yield from dma_unit(wq_d, 2, "q2")
                yield from proj_units("q2", qt, 2 * T)
                yield from dma_unit(wq_d, 1, "q1")
                yield from proj_units("q1", qt, T)
                yield from dma_unit(wk_d, 1, "k1")
                yield from proj_units("k1", kt, T)
                yield from dma_unit(wv_d, 1, "v1")
                yield from proj_v_units(
                    1, wpool.tile([128, T], bf16, name="vts_1", tag="w"))
                yield from dma_unit(wq_d, 3, "q3")
                # t4 order (1,2,3,0): head 3 runs blocks [1,2,3,0], so the
                # final sub-strip (t4=0) feeds the final block and its units
                # stream through the second-to-last block's yields instead
                # of leaving the endgame with no filler.
                yield from proj_units("q3", qt, 3 * T, t4s=(1, 2, 3, 0))

            filler.add(chain_rest())

            # forced-drain unit counts (cumulative filler indices).
            # v0 rest: t4=1..3 interleaved with vtr0 groups, 10 units per
            # t4; vtr0 group b done at unit 10*b (b>=1).
            U_V0 = 30
            U_Q2 = [U_V0 + 1 + 9 * (t + 1) for t in range(4)]
            U_Q1 = U_Q2[3] + 1 + 36
            U_K1 = [U_Q1 + 1 + 9 * (t + 1) for t in range(4)]
            U_V1 = [U_K1[3] + 1 + 10 * (t + 1) for t in range(4)]
            base3 = U_V1[3] + 1
            U_Q3 = [base3 + 36, base3 + 9, base3 + 18, base3 + 27]
            N_UNITS = U_Q3[0]

            # attention blocks in dependency order, with forced drains;
            # background pump keeps filler flowing evenly (ratio of units
            # per yield).
            blocks = []
            for b in range(4):
                blocks.append((0, b, 10 * b))   # a0bX needs vtr0 group X
            for b in range(4):
                blocks.append((2, b, U_Q2[b]))
            for b in range(4):
                blocks.append((1, b, max(U_K1[b], U_V1[b])))
            # head 3 ends the kernel with its smallest block (b=0, whose
            # strip data drained long before): ~2us of exp vs ~8us for a
            # b=3 block, minimizing the ACT-over-PE exposure at the tail.
            for b in (1, 2, 3, 0):
                blocks.append((3, b, U_Q3[b]))
            last_block[0] = (3, 0)

            # stream filler gradually: during block i's yields, pump
            # linearly from its own need toward the next block's need, so
            # forced drains never burst and the endgame blocks keep filler.
            needs = [need for _, _, need in blocks] + [N_UNITS]
            for i, (h, b, need) in enumerate(blocks):
                filler.pump_to(need)
                nxt = max(needs[i + 1], need)
                ny = 2 * (4 * b + 4) + 2
                k = 0
                for _ in attn_block(h, b):
                    k += 1
                    filler.pump_to(need + (nxt - need) * k // ny)
            filler.drain()

    nc.compile()
    _prog_cache["nc"] = nc
    return nc


def _host_prep(x, Wq, bq, Wk, bk, Wv, bv):
    """Shard + repack inputs for the 8 cores. Returns in_maps list."""
    assert x.shape == (1, T, C)
    assert np.abs(bq).max() == 0 and np.abs(bk).max() == 0, \
        "nonzero q/k biases not supported"

    x0 = np.ascontiguousarray(x[0]).astype(BF16)
    # xT packed chunk-major: [kc, p, t] = x[t, 128*kc + p] -- each chunk is
    # a contiguous 512KB block so its DMA descriptor is dense (cheap issue)
    xT = np.ascontiguousarray(x0.reshape(T, KC, 128).transpose(1, 2, 0))

    # causal diag-tile mask tri[tk, tq] = tq >= tk, plus identity
    tq = np.arange(128)[None, :]
    tk = np.arange(128)[:, None]
    masks = np.concatenate(
        [(tq >= tk).astype(BF16), np.eye(128, dtype=BF16)], axis=1)
    masks = np.ascontiguousarray(masks)

    def pack_w(Wrows):
        # Wrows: [128 (out c), C (in)] for one head ->
        # packed[p, 128*kc + c] = Wrows[c, 128*kc + p]
        return np.ascontiguousarray(
            Wrows.astype(BF16).reshape(128, KC, 128).transpose(2, 1, 0)
            .reshape(128, C))

    in_maps = []
    for c in range(N_CORES):
        qheads = [2 * c, 2 * c + 1, 2 * c + 16, 2 * c + 17]
        kvheads = [2 * c, 2 * c + 1]
        wq = np.stack([pack_w(Wq[128 * H:128 * (H + 1)]) for H in qheads])
        wk = np.stack([pack_w(Wk[128 * K:128 * (K + 1)]) for K in kvheads])
        wv = np.stack([pack_w(Wv[128 * K:128 * (K + 1)]) for K in kvheads])
        in_maps.append({
            "xT": xT, "wq": wq, "wk": wk, "wv": wv, "masks": masks,
        })
    return in_maps


def _assemble(results, bv):
    out = np.empty((T, C), dtype=np.float32)
    for c in range(N_CORES):
        r = results[c]["out"]
        qheads = [2 * c, 2 * c + 1, 2 * c + 16, 2 * c + 17]
        for i, H in enumerate(qheads):
            blk = r[:, 128 * i:128 * (i + 1)]
            if bv is not None:
                blk = blk + bv[128 * (H % N_KV_IDX):128 * (H % N_KV_IDX) + 128]
            out[:, 128 * H:128 * (H + 1)] = blk
    return out.reshape(1, T, C)


N_KV_IDX = 16


def _install_trace_hooks():
    """The agent image's antenv lacks axon_hooks; recreate it so
    run_bass_kernel_spmd's trace=True path can capture NTFF profiles."""
    import sys
    import types
    import antenv
    if "antenv.axon_hooks" not in sys.modules:
        mod = types.ModuleType("antenv.axon_hooks")
        mod._hook = None

        def set_axon_ntff_profile_hook(h):
            mod._hook = h

        def get_axon_ntff_profile_hook():
            return mod._hook

        mod.set_axon_ntff_profile_hook = set_axon_ntff_profile_hook
        mod.get_axon_ntff_profile_hook = get_axon_ntff_profile_hook
        sys.modules["antenv.axon_hooks"] = mod
        antenv.axon_hooks = mod
    from antenv.axon_hooks import (get_axon_ntff_profile_hook,
                                   set_axon_ntff_profile_hook)
    if get_axon_ntff_profile_hook() is None:
        if "/root/.axon_site" not in sys.path:
            sys.path.insert(0, "/root/.axon_site")
        from trn_agent_boot.trn_boot import _ntff_profile_via_ctypes
        set_axon_ntff_profile_hook(
            _ntff_profile_via_ctypes("/opt/axon/libaxon_pjrt.so"))
    import concourse.bass_utils as bu
    bu.upload_artifacts = lambda tmpdir: tmpdir


def _run(inputs, trace=False, trace_kwargs=None):
    if trace:
        _install_trace_hooks()
    from concourse.bass_utils import run_bass_kernel_spmd
    nc = _build_program()
    in_maps = _host_prep(**inputs)
    res = run_bass_kernel_spmd(
        nc, in_maps, list(range(N_CORES)), trace=trace,
        **(trace_kwargs or {}))
    bv = inputs["bv"].astype(np.float32)
    bv = bv if np.abs(bv).max() > 0 else None
    out = _assemble(res.results, bv)
    return out, res


def kernel(x, Wq, bq, Wk, bk, Wv, bv):
    out, _ = _run(dict(x=np.asarray(x), Wq=np.asarray(Wq), bq=np.asarray(bq),
                       Wk=np.asarray(Wk), bk=np.asarray(bk),
                       Wv=np.asarray(Wv), bv=np.asarray(bv)))
    return out


# revision 19
# speedup vs baseline: 1.0179x; 1.0013x over previous
"""GQA kernel for Trainium2, 8 NeuronCores, tensor-parallel over heads.

Problem: B=1, T=2048, C=4096, 32 q-heads, 16 kv-heads, head_dim=128,
scale = 1/sqrt(32), causal. q head H uses kv head H%16.

Sharding (no collectives needed): core c owns q-heads
{2c, 2c+1, 2c+16, 2c+17} and kv-heads {2c, 2c+1}. Each output column
block depends only on its own head, so the full output is a host-side
concat of per-core column slices.

Per-core kernel (all matmuls bf16, fp32 PSUM accumulation):
  xT resident in SBUF as [C=4096 (32 chunks of 128 part), T=2048].
  qT[h] = Wq_h @ xT    -> [128 (D), 2048 (T)]   (4 heads)
  kT[kv] = Wk_kv @ xT  -> [128 (D), 2048 (T)]   (2 kv heads)
  v[kv]  = x @ Wv_kv^T -> [2048 (T part), 128+1] (ones col for row sums)
  S^T tile (per 128-wide Tk tile j) = kT_j^T @ qT_block -> [128,512] PSUM
  P^T = exp(SCALE * S^T) (ACT), causal mask via {0,1} multiply on the
        true-diagonal 128-wide tile only; diagonal tiles narrowed to
        the live Tq columns.
  out[Tq,128] (+ row-sum col) = sum_j P^T_j.T @ v_j  (PSUM accum,
        4 Tq-chunks of 128 packed 2-per-bank, zeroed by a PE matmul)
  out normalized by reciprocal(row sum) (DVE), DMA'd out fp32.

Schedule (the big idea vs the 335us version): attention is paced by
the Scalar/ACT engine's exp (~21.6us per head vs ~19us of PE work),
and projections are paced by the PE with ACT idle. Interleaving them
hides the exp entirely: all post-phase-A projection strips are chopped
into 4-kc (0.85us) filler units and pumped into the PE queue between
each attention j-tile's S and PV, via one global filler queue with
forced drains at dependency points (block (h,b) forces its qt/kt/vt
producers). The 150-matmul HAM warmup is gone: phase A's real matmuls
start at first-DMA-arrival and absorb the clock ramp.
"""

import numpy as np
import ml_dtypes

BF16 = ml_dtypes.bfloat16
T = 2048
C = 4096
D = 128
N_HEADS = 32
N_KV = 16
SCALE = float(1.0 / np.sqrt(np.float32(N_HEADS)))
KC = C // 128          # 32 contraction chunks
NQH = 4                # local q heads per core
NKV = 2                # local kv heads per core
NT = T // 128          # 16 token tiles
VROW = D + 1           # 129: v with ones column
N_CORES = 8

_prog_cache = {}


def _build_program():
    if "nc" in _prog_cache:
        return _prog_cache["nc"]
    import concourse.bass as bass
    import concourse.tile as tile
    from concourse import bacc, mybir

    dt = mybir.dt
    f32 = dt.float32
    bf16 = dt.bfloat16
    EXP = mybir.ActivationFunctionType.Exp
    COPY = mybir.ActivationFunctionType.Copy

    nc = bacc.Bacc("TRN2", target_bir_lowering=False, debug=False,
                   num_devices=N_CORES)

    xT_d = nc.dram_tensor("xT", [KC, 128, T], bf16,
                          kind="ExternalInput").ap()
    wq_d = nc.dram_tensor("wq", [NQH, 128, C], bf16, kind="ExternalInput").ap()
    wk_d = nc.dram_tensor("wk", [NKV, 128, C], bf16, kind="ExternalInput").ap()
    wv_d = nc.dram_tensor("wv", [NKV, 128, C], bf16, kind="ExternalInput").ap()
    # masks: [128,128] causal diag tile + [128,128] identity for PE transpose
    mask_d = nc.dram_tensor("masks", [128, 256], bf16,
                            kind="ExternalInput").ap()
    out_d = nc.dram_tensor("out", [T, NQH * D], f32, kind="ExternalOutput").ap()

    with tile.TileContext(nc) as tc:
        with (
            tc.tile_pool(name="persist", bufs=1) as persist,
            tc.tile_pool(name="xpool", bufs=1) as xpool,
            tc.tile_pool(name="wpool", bufs=3) as wpool,
            tc.tile_pool(name="ptpool", bufs=3) as ptpool,
            tc.tile_pool(name="opool", bufs=4) as opool,
            tc.tile_pool(name="recpool", bufs=4) as recpool,
            # PSUM: 8 banks = pv 4x[128,512] + spp 2x[128,512] + proj 2x[128,512]
            tc.tile_pool(name="psum", bufs=1, space=bass.MemorySpace.PSUM) as psum,
        ):
            mask_sb = persist.tile([128, 256], bf16, name="mask_sb",
                                   tag="mask_sb")
            tri = mask_sb[:, 0:128]
            ident = mask_sb[:, 128:256]

            qt = persist.tile([128, NQH * T], bf16, name="qt", tag="qt")
            kt = persist.tile([128, NKV * T], bf16, name="kt", tag="kt")
            vt = persist.tile([128, NKV * NT * VROW], bf16, name="vt", tag="vt")
            zer = persist.tile([128, 128], bf16, name="zer", tag="zer")
            nc.vector.memset(zer[:], 0.0)

            # ones columns of v (row-sum trick)
            for i in range(NKV * NT):
                nc.vector.memset(vt[:, i * VROW + D: (i + 1) * VROW], 1.0)

            # xT is DMA'd as 32 per-kc chunks [128, T] so the startup
            # projections can consume chunks at DMA arrival pace.
            xts = [None] * KC
            wts = {}

            def dma_w(src, idx, key):
                w = wpool.tile([128, C], bf16, name=f"w_{key}", tag="w")
                nc.sync.dma_start(out=w[:], in_=src[idx])
                wts[key] = w

            def dma_x(kc):
                xt = xpool.tile([128, T], bf16, name=f"xt{kc}", tag=f"xt{kc}")
                nc.sync.dma_start(out=xt[:], in_=xT_d[kc])
                xts[kc] = xt

            def xs(kc, lo, size):
                return xts[kc][:, lo: lo + size]

            def ps_tile(tag, name, shape=(128, 512), dtyp=f32, bufs=None):
                return psum.tile(list(shape), dtyp, name=name, tag=tag,
                                 bufs=bufs)

            def proj_phase_a():
                """q0+k0 strips, kc-outer so program order matches x-chunk
                DMA arrival: PE does 8 matmuls (4.1K cycles) per 0.5MB chunk
                (~1.2us DMA), staying busy through the whole x load. Runs
                straight out of reset: the first couple kc's absorb the HAM
                clock ramp while the DMA stream is still the pacer."""
                wq = wts.pop("q0")
                wk = wts.pop("k0")
                psq = [ps_tile("pv", f"psA_q{t}", bufs=4)[:] for t in range(4)]
                psk = ([ps_tile("proj", f"psA_k{t}", bufs=2)[:]
                        for t in range(2)]
                       + [ps_tile("spp", f"psA_k{t + 2}", bufs=2)[:]
                          for t in range(2)])
                with nc.named_scope("phaseA"):
                    for kc in range(KC):
                        for t4 in range(4):
                            nc.tensor.matmul(
                                psq[t4], lhsT=wq[:, kc * 128:(kc + 1) * 128],
                                rhs=xs(kc, t4 * 512, 512),
                                start=(kc == 0), stop=(kc == KC - 1))
                        for t4 in range(4):
                            nc.tensor.matmul(
                                psk[t4], lhsT=wk[:, kc * 128:(kc + 1) * 128],
                                rhs=xs(kc, t4 * 512, 512),
                                start=(kc == 0), stop=(kc == KC - 1))
                    # cast order: k t4=0,1 first (frees the proj bufs for the
                    # v0 strip), then q (unblocks attn0's S), then k t4=2,3
                    # (frees the spp bufs). Alternate DVE/ACT to halve the
                    # chain.
                    def cast(dst, src, on_act):
                        if on_act:
                            nc.scalar.copy(out=dst, in_=src)
                        else:
                            nc.vector.tensor_copy(out=dst, in_=src)

                    cast(kt[:, 0:512], psk[0], False)
                    cast(kt[:, 512:1024], psk[1], True)
                    for t4 in range(4):
                        cast(qt[:, t4 * 512:(t4 + 1) * 512], psq[t4],
                             t4 % 2 == 1)
                    cast(kt[:, 1024:1536], psk[2], False)
                    cast(kt[:, 1536:2048], psk[3], True)

            # ---- filler machinery -------------------------------------
            # Projection strips (and their DMAs / v-transposes) are chopped
            # into ~0.85us units, pumped between attention emissions.
            class Filler:
                def __init__(self):
                    self.gens = []
                    self.emitted = 0

                def add(self, gen):
                    self.gens.append(gen)

                def pump(self, n):
                    done = 0
                    while done < n and self.gens:
                        try:
                            next(self.gens[0])
                            done += 1
                            self.emitted += 1
                        except StopIteration:
                            self.gens.pop(0)
                    return done

                def pump_to(self, total):
                    if total > self.emitted:
                        self.pump(total - self.emitted)

                def drain(self):
                    self.pump(1 << 30)

            filler = Filler()

            def dma_unit(src, idx, key):
                dma_w(src, idx, key)
                yield

            def proj_units(key, dest, dbase, t4s=(0, 1, 2, 3)):
                """[D, T] projection strip as filler units: 8 matmul units
                + 1 cast unit per 512-wide Tq block."""
                w = wts[key]
                for t4 in t4s:
                    ps = ps_tile("proj", f"ps_{key}_{t4}", bufs=2)[:]
                    for g in range(8):
                        for kc in range(g * 4, (g + 1) * 4):
                            nc.tensor.matmul(
                                ps,
                                lhsT=w[:, kc * 128:(kc + 1) * 128],
                                rhs=xs(kc, t4 * 512, 512),
                                start=(kc == 0), stop=(kc == KC - 1),
                            )
                        yield
                    nc.vector.tensor_copy(
                        out=dest[:, dbase + t4 * 512: dbase + (t4 + 1) * 512],
                        in_=ps)
                    yield

            def vtr_units(kv, vts, groups=(0, 1, 2, 3)):
                """PE-transpose the [D,T] v strip into vt [Tk,D] tiles,
                one 4-tile group (one Tq block) per unit."""
                for g in groups:
                    for m in range(g * 4, (g + 1) * 4):
                        ps = ps_tile("proj", f"ps_vt_{kv}_{m}",
                                     shape=(128, 128), dtyp=bf16, bufs=2)
                        nc.tensor.transpose(
                            ps[:], vts[:, m * 128:(m + 1) * 128], ident)
                        nc.vector.tensor_copy(
                            out=vt[:, (kv * NT + m) * VROW:
                                   (kv * NT + m) * VROW + D],
                            in_=ps[:])
                    yield

            def proj_v_units(kv, vts, t4s=(0, 1, 2, 3)):
                """v strip with its transposes interleaved: [9 strip units,
                1 vtr unit] per t4, so vt tiles for Tq block X are ready
                10*(X+1) units in."""
                pg = proj_units(f"v{kv}", vts, 0, t4s)
                vg = vtr_units(kv, vts, t4s)
                for _ in t4s:
                    for _ in range(9):
                        next(pg)
                        yield
                    next(vg)
                    yield

            # ---- attention --------------------------------------------
            last_block = [None]  # (h, b) of the final block, for tail split

            def attn_block(h, b):
                """One [512 Tq] block of head h; yields at filler points."""
                kv = h % 2
                qblk = qt[:, h * T + b * 512: h * T + (b + 1) * 512]
                # pv outs are VROW=129 fp32: pack two per PSUM bank
                # (offsets 0/256) so a block holds 2 of the 4 "pv" bufs
                # (double-buffered across blocks). The first PV matmul of
                # each group (j=4b, executed first) carries start=True.
                pvt = [ps_tile("pv", f"pv_{h}_{b}_{i}", bufs=4)
                       for i in range(2)]

                def pv_ap(s, lo, hi):
                    base = 256 * (s % 2)
                    return pvt[s // 2][:, base + lo: base + hi]

                # diag tiles first: their longer exp->mask->PV chain
                # overlaps filler instead of forming the block's tail.
                j_list = list(range(4 * b, 4 * b + 4)) + list(range(4 * b))
                first = True
                prev = None  # deferred (j, pt) whose PV is pending
                for j in j_list:
                    r = j - 4 * b  # >=0 on diagonal tiles
                    roff = max(0, r) * 128
                    spp = ps_tile("spp", f"sp_{h}_{b}_{j}", bufs=2)
                    nc.tensor.matmul(
                        spp[:, roff:512],
                        lhsT=kt[:, kv * T + j * 128: kv * T + (j + 1) * 128],
                        rhs=qblk[:, roff:512],
                        start=True, stop=True,
                    )
                    # alternate tags -> consecutive j land in non-adjacent
                    # SBUF slots, preventing the backend from fusing
                    # consecutive exps into one wide ACTIVATE.
                    pt = ptpool.tile([128, 512], bf16, name=f"pt_{h}_{b}_{j}",
                                     tag=f"pt{j % 2}")
                    nc.scalar.activation(pt[:, roff:512], spp[:, roff:512],
                                         EXP, scale=SCALE)
                    if r >= 0:
                        # true-diagonal 128-wide tile needs masking; PV for
                        # s > r proceeds straight after exp.
                        nc.vector.tensor_mul(
                            pt[:, roff: roff + 128],
                            pt[:, roff: roff + 128], tri)
                    yield
                    if first:
                        first = False
                        # zero the packed pv banks via PE (zeros
                        # stationary): start=True clears the FULL bank on
                        # TRN2 (measured), so packed groups must zero via
                        # matmul (order-independent) and pure-accumulate.
                        for i in range(2):
                            for base in (0, 256):
                                nc.tensor.matmul(
                                    pvt[i][:, base: base + VROW],
                                    lhsT=zer[:], rhs=qblk[:, 0:VROW],
                                    start=True, stop=False,
                                    skip_group_check=True)
                        yield
                    tail = (h, b) == last_block[0]
                    if prev is not None:
                        emit_pv(h, b, prev[0], prev[1], pv_ap)
                        if tail:
                            # terminal block (b=0): group s finishes at
                            # j=s, so normalize+DMA each group as soon as
                            # its accumulation closes, shortening the
                            # kernel tail to just the last group's chain.
                            norm_group(h, b, prev[0], pv_ap, prev[0] % 2)
                        yield
                    prev = (j, pt)
                emit_pv(h, b, prev[0], prev[1], pv_ap)
                if tail:
                    norm_group(h, b, prev[0], pv_ap, 1)
                    yield
                else:
                    yield
                    # normalize into one [128, 512] staging tile, single
                    # DMA for the whole block (4 separate 64KB DMAs
                    # serialize on the queue and stretch the tail). Both
                    # APs keep the partition dim leading.
                    ot = opool.tile([128, 512], f32, name=f"ot_{h}_{b}",
                                    tag="ot")
                    for s in range(4):
                        rec = recpool.tile([128, 1], f32,
                                           name=f"rec_{h}_{b}_{s}", tag="rec")
                        nc.vector.reciprocal(rec[:], pv_ap(s, D, D + 1))
                        nc.vector.tensor_scalar_mul(
                            ot[:, s * 128:(s + 1) * 128], pv_ap(s, 0, D),
                            rec[:])
                    nc.sync.dma_start(
                        out=out_d[b * 512:(b + 1) * 512, h * D:(h + 1) * D]
                        .rearrange("(s p) d -> p s d", s=4),
                        in_=ot[:].rearrange("p (s d) -> p s d", s=4))
                    yield

            def norm_group(h, b, s, pv_ap, on_act):
                rec = recpool.tile([128, 1], f32,
                                   name=f"rec_{h}_{b}_{s}", tag="rec")
                nc.vector.reciprocal(rec[:], pv_ap(s, D, D + 1))
                ot = opool.tile([128, 128], f32, name=f"otg_{h}_{b}_{s}",
                                tag="otg")
                if on_act:
                    nc.scalar.activation(ot[:], pv_ap(s, 0, D), COPY,
                                         scale=rec[:])
                else:
                    nc.vector.tensor_scalar_mul(ot[:], pv_ap(s, 0, D), rec[:])
                nc.sync.dma_start(
                    out=out_d[b * 512 + s * 128: b * 512 + (s + 1) * 128,
                              h * D:(h + 1) * D],
                    in_=ot[:])

            def emit_pv(h, b, j, pt, pv_ap):
                r = j - 4 * b
                kv = h % 2
                vsl = vt[:, (kv * NT + j) * VROW: (kv * NT + j + 1) * VROW]
                # exec order: diag js (4b..4b+3) first, then off-diag
                # 0..4b-1 -> stop on the last executed contribution.
                for s in range(max(0, r), 4):
                    nc.tensor.matmul(
                        pv_ap(s, 0, VROW),
                        lhsT=pt[:, s * 128: (s + 1) * 128],
                        rhs=vsl,
                        start=False,
                        stop=(j == ((4 * b - 1) if b > 0 else s)),
                        skip_group_check=True,
                    )

            # ---- DMA schedule -----------------------------------------
            # wq0/wk0 in pieces interleaved ahead of the x chunks they
            # gate; the first pieces are a single kc (64KB) so the first
            # matmul's gate (wq kc0 + x chunk 0) clears as early as
            # possible. masks deferred (first needed by vtr0, ~30us after
            # the x load completes).
            wq0 = wpool.tile([128, C], bf16, name="w_q0", tag="w")
            wk0 = wpool.tile([128, C], bf16, name="w_k0", tag="w")
            wts["q0"] = wq0
            wts["k0"] = wk0

            nc.sync.dma_start(out=wq0[:, 0:128], in_=wq_d[0][:, 0:128])
            xt0 = xpool.tile([128, T], bf16, name="xt0", tag="xt0")
            xts[0] = xt0
            nc.sync.dma_start(out=xt0[:, 0:512], in_=xT_d[0][:, 0:512])
            nc.sync.dma_start(out=wk0[:, 0:128], in_=wk_d[0][:, 0:128])
            nc.sync.dma_start(out=xt0[:, 512:2048], in_=xT_d[0][:, 512:2048])
            nc.sync.dma_start(out=wq0[:, 128:512], in_=wq_d[0][:, 128:512])
            dma_x(1)
            nc.sync.dma_start(out=wk0[:, 128:512], in_=wk_d[0][:, 128:512])
            dma_x(2)
            dma_x(3)
            for piece in range(1, 8):
                sl = slice(piece * 512, (piece + 1) * 512)
                nc.sync.dma_start(out=wq0[:, sl], in_=wq_d[0][:, sl])
                dma_x(piece * 4)
                nc.sync.dma_start(out=wk0[:, sl], in_=wk_d[0][:, sl])
                for kc in range(piece * 4 + 1, (piece + 1) * 4):
                    dma_x(kc)
            dma_w(wv_d, 0, "v0")
            nc.sync.dma_start(out=mask_sb[:], in_=mask_d[:])

            # ---- emission ---------------------------------------------
            # HAM warm-up: the PE is data-starved until ~10us (DMA queue
            # bring-up + first x chunk) and its clock gate opens slowly;
            # a short burst of dummy matmuls in the dead window opens the
            # gate before real work arrives, without delaying it.
            wps = ps_tile("pv", "warm", bufs=4)
            for _ in range(64):
                nc.tensor.matmul(wps[:, 0:128], lhsT=zer[:], rhs=zer[:],
                                 start=True, stop=True)
            proj_phase_a()

            # v0 head: t4=0 strip + its transposes run pure (nothing to
            # overlap yet); the rest becomes filler for attn0.
            vts0 = wpool.tile([128, T], bf16, name="vts_0", tag="w")
            for _ in proj_v_units(0, vts0, t4s=(0,)):
                pass

            # global filler queue: remaining v0 + all later strips.
            def chain_rest():
                yield from proj_v_units(0, vts0, t4s=(1, 2, 3))
                yield from dma_unit(wq_d, 2, "q2")
                yield from proj_units("q2", qt, 2 * T)
                yield from dma_unit(wq_d, 1, "q1")
                yield from proj_units("q1", qt, T)
                yield from dma_unit(wk_d, 1, "k1")
                yield from proj_units("k1", kt, T)
                yield from dma_unit(wv_d, 1, "v1")
                yield from proj_v_units(
                    1, wpool.tile([128, T], bf16, name="vts_1", tag="w"))
                yield from dma_unit(wq_d, 3, "q3")
                # t4 order (1,2,3,0): head 3 runs blocks [1,2,3,0], so the
                # final sub-strip (t4=0) feeds the final block and its units
                # stream through the second-to-last block's yields instead
                # of leaving the endgame with no filler.
                yield from proj_units("q3", qt, 3 * T, t4s=(1, 2, 3, 0))

            filler.add(chain_rest())

            # forced-drain unit counts (cumulative filler indices).
            # v0 rest: t4=1..3 interleaved with vtr0 groups, 10 units per
            # t4; vtr0 group b done at unit 10*b (b>=1).
            U_V0 = 30
            U_Q2 = [U_V0 + 1 + 9 * (t + 1) for t in range(4)]
            U_Q1 = U_Q2[3] + 1 + 36
            U_K1 = [U_Q1 + 1 + 9 * (t + 1) for t in range(4)]
            U_V1 = [U_K1[3] + 1 + 10 * (t + 1) for t in range(4)]
            base3 = U_V1[3] + 1
            U_Q3 = [base3 + 36, base3 + 9, base3 + 18, base3 + 27]
            N_UNITS = U_Q3[0]

            # attention blocks in dependency order, with forced drains;
            # background pump keeps filler flowing evenly (ratio of units
            # per yield).
            blocks = []
            for b in range(4):
                blocks.append((0, b, 10 * b))   # a0bX needs vtr0 group X
            for b in range(4):
                blocks.append((2, b, U_Q2[b]))
            for b in range(4):
                blocks.append((1, b, max(U_K1[b], U_V1[b])))
            # head 3 ends the kernel with its smallest block (b=0, whose
            # strip data drained long before): ~2us of exp vs ~8us for a
            # b=3 block, minimizing the ACT-over-PE exposure at the tail.
            for b in (1, 2, 3, 0):
                blocks.append((3, b, U_Q3[b]))
            last_block[0] = (3, 0)

            # stream filler gradually: during block i's yields, pump
            # linearly from its own need toward the next block's need, so
            # forced drains never burst and the endgame blocks keep filler.
            needs = [need for _, _, need in blocks] + [N_UNITS]
            for i, (h, b, need) in enumerate(blocks):
                filler.pump_to(need)
                nxt = max(needs[i + 1], need)
                ny = 2 * (4 * b + 4) + 2
                k = 0
                for _ in attn_block(h, b):
                    k += 1
                    filler.pump_to(need + (nxt - need) * k // ny)
            filler.drain()

    nc.compile()
    _prog_cache["nc"] = nc
    return nc


def _host_prep(x, Wq, bq, Wk, bk, Wv, bv):
    """Shard + repack inputs for the 8 cores. Returns in_maps list."""
    assert x.shape == (1, T, C)
    assert np.abs(bq).max() == 0 and np.abs(bk).max() == 0, \
        "nonzero q/k biases not supported"

    x0 = np.ascontiguousarray(x[0]).astype(BF16)
    # xT packed chunk-major: [kc, p, t] = x[t, 128*kc + p] -- each chunk is
    # a contiguous 512KB block so its DMA descriptor is dense (cheap issue)
    xT = np.ascontiguousarray(x0.reshape(T, KC, 128).transpose(1, 2, 0))

    # causal diag-tile mask tri[tk, tq] = tq >= tk, plus identity
    tq = np.arange(128)[None, :]
    tk = np.arange(128)[:, None]
    masks = np.concatenate(
        [(tq >= tk).astype(BF16), np.eye(128, dtype=BF16)], axis=1)
    masks = np.ascontiguousarray(masks)

    def pack_w(Wrows):
        # Wrows: [128 (out c), C (in)] for one head ->
        # packed[p, 128*kc + c] = Wrows[c, 128*kc + p]
        return np.ascontiguousarray(
            Wrows.astype(BF16).reshape(128, KC, 128).transpose(2, 1, 0)
            .reshape(128, C))

    in_maps = []
    for c in range(N_CORES):
        qheads = [2 * c, 2 * c + 1, 2 * c + 16, 2 * c + 17]
        kvheads = [2 * c, 2 * c + 1]
        wq = np.stack([pack_w(Wq[128 * H:128 * (H + 1)]) for H in qheads])
        wk = np.stack([pack_w(Wk[128 * K:128 * (K + 1)]) for K in kvheads])
        wv = np.stack([pack_w(Wv[128 * K:128 * (K + 1)]) for K in kvheads])
        in_maps.append({
            "xT": xT, "wq": wq, "wk": wk, "wv": wv, "masks": masks,
        })
    return in_maps


def _assemble(results, bv):
    out = np.empty((T, C), dtype=np.float32)
    for c in range(N_CORES):
        r = results[c]["out"]
        qheads = [2 * c, 2 * c + 1, 2 * c + 16, 2 * c + 17]
        for i, H in enumerate(qheads):
            blk = r[:, 128 * i:128 * (i + 1)]
            if bv is not None:
                blk = blk + bv[128 * (H % N_KV_IDX):128 * (H % N_KV_IDX) + 128]
            out[:, 128 * H:128 * (H + 1)] = blk
    return out.reshape(1, T, C)


N_KV_IDX = 16


def _install_trace_hooks():
    """The agent image's antenv lacks axon_hooks; recreate it so
    run_bass_kernel_spmd's trace=True path can capture NTFF profiles."""
    import sys
    import types
    import antenv
    if "antenv.axon_hooks" not in sys.modules:
        mod = types.ModuleType("antenv.axon_hooks")
        mod._hook = None

        def set_axon_ntff_profile_hook(h):
            mod._hook = h

        def get_axon_ntff_profile_hook():
            return mod._hook

        mod.set_axon_ntff_profile_hook = set_axon_ntff_profile_hook
        mod.get_axon_ntff_profile_hook = get_axon_ntff_profile_hook
        sys.modules["antenv.axon_hooks"] = mod
        antenv.axon_hooks = mod
    from antenv.axon_hooks import (get_axon_ntff_profile_hook,
                                   set_axon_ntff_profile_hook)
    if get_axon_ntff_profile_hook() is None:
        if "/root/.axon_site" not in sys.path:
            sys.path.insert(0, "/root/.axon_site")
        from trn_agent_boot.trn_boot import _ntff_profile_via_ctypes
        set_axon_ntff_profile_hook(
            _ntff_profile_via_ctypes("/opt/axon/libaxon_pjrt.so"))
    import concourse.bass_utils as bu
    bu.upload_artifacts = lambda tmpdir: tmpdir


def _run(inputs, trace=False, trace_kwargs=None):
    if trace:
        _install_trace_hooks()
    from concourse.bass_utils import run_bass_kernel_spmd
    nc = _build_program()
    in_maps = _host_prep(**inputs)
    res = run_bass_kernel_spmd(
        nc, in_maps, list(range(N_CORES)), trace=trace,
        **(trace_kwargs or {}))
    bv = inputs["bv"].astype(np.float32)
    bv = bv if np.abs(bv).max() > 0 else None
    out = _assemble(res.results, bv)
    return out, res


def kernel(x, Wq, bq, Wk, bk, Wv, bv):
    out, _ = _run(dict(x=np.asarray(x), Wq=np.asarray(Wq), bq=np.asarray(bq),
                       Wk=np.asarray(Wk), bk=np.asarray(bk),
                       Wv=np.asarray(Wv), bv=np.asarray(bv)))
    return out
